# revision 34
# baseline (speedup 1.0000x reference)
"""AttnBlock (GroupNorm -> QKV 1x1 -> full attention over 1024 tokens -> out-proj
+ residual) for x [32, 512, 32, 32] f32, distributed data-parallel over 8
NeuronCores (4 samples per core, weights replicated).

Per-core single-NC Bass/Tile kernel, v2: weight-fusion + all-fp8 DoubleRow.

Algebraic restructuring (host-side, exact in f32):
  - scores  = (Wq h)^T (Wk h) = h^T A h with A = Wq @ Wk.T  -> ONE fused
    "qc" projection (qc = A^T h) replaces the separate Q and K projections.
  - out-proj fused into V: vt' = (Wv @ Wt)^T h gives
    out = vt'^T P_norm  directly, so the PV matmul's PSUM already holds the
    final pre-residual activation -- the separate out-projection disappears.
  - A and B=Wv@Wt are pre-scaled by 8 on the host so their entries clear the
    fp8e4 subnormal range; the 8x cancels via exp-scale (CINV/8) on the
    score side and via rep = 1/(8*den) on the PV side.

Per-sample PE work drops 82944 -> ~50200 col-cycles (scores 16384 + PV-DR
16384 + qc 8192 + vt' 8192 + den 1024): every matmul issues moving columns at
1/cycle regardless of dtype, so fp8 DoubleRow's 2x comes purely from halving
the pass count (contraction 256 rows/pass).

  - est = exp(s*c - 4.5) stored fp8e5 (e5m2: scores empirically reach 13.8
    sigma, far past fp8e4's e^11.7 dynamic range; e5m2 spans it easily and
    softmax normalization cancels most of its 2-mantissa-bit noise -- host
    emulation: 0.0078 rel err vs 0.0068 for bf16 est). The offset cancels
    exactly in softmax. PV runs fp8 DoubleRow (e4m3 vt x e5m2 est) over
    jm-pair passes.
  - softmax denominator: DVE pairwise-add tree over the 8 fp8 exp tiles
    (bf16 out, 2x DVE rate), one 8.0-matrix bf16 matmul reduces partitions
    and replicates 8*den; rep = 1/(8 den) via reciprocal_approx_fast.
  - residual: tmp = PV_psum * rep on DVE, out = tmp + x on GpSimd (idle
    otherwise; DVE for the last sample to shorten the tail), DMA per half.
  - engine balance per sample: PE ~20.9us, DVE ~19.8us (GN stats/apply, den
    tree, recip, tmp), ACT ~17.4us (exp + qc/vt' psum->sbuf fp8 copies),
    GpSimd ~9us (residual adds + out DMA issue).
  - prologue: x[0] owns all three DMA rings before weights/x[1..3] queue up;
    GN(0) stats start per-tile as x[0] tiles land.
"""

import os
import sys

import numpy as np

sys.path.insert(0, "/opt/trn_rl_repo")

import ml_dtypes  # noqa: E402

import concourse.bass as bass  # noqa: E402
import concourse.tile as tile  # noqa: E402
from concourse import bacc, mybir  # noqa: E402

P = 128
B_FULL, C, H, W = 32, 512, 32, 32
HW = H * W            # 1024 tokens
N_CORES = 8
NB = B_FULL // N_CORES  # 4 samples per core
NT = C // P           # 4 channel tiles
NP = NT // 2          # 2 DoubleRow channel-tile pairs
NJ = HW // P          # 8 token tiles
NJP = NJ // 2         # 4 DoubleRow token-tile pairs
NGROUPS = 32
GS = C // NGROUPS     # 16 channels per group
EPS = 1e-6
CINV = float(C) ** -0.5
WSC = 8.0             # fused weights pre-scaled by 8 (fp8 subnormal guard)
EOFF = -4.5           # exp offset: est = exp(s*c - 4.5), cancels in softmax

f32 = mybir.dt.float32
bf16 = mybir.dt.bfloat16
f8 = mybir.dt.float8e4
f8e5 = mybir.dt.float8e5
ALU = mybir.AluOpType
ACT = mybir.ActivationFunctionType
DR = mybir.MatmulPerfMode.DoubleRow


def build_nc(zero_out_bias=True):
    """Build the single-core Bass graph (SPMD: same graph on all 8 cores).

    zero_out_bias: fused output bias bt' = bt + Wt^T bv is all-zero (true for
    this problem), so the residual add drops the bias column.
    """
    nc = bacc.Bacc("TRN2", target_bir_lowering=False, debug=False)

    x_d = nc.dram_tensor("x", [NB, C, HW], f32, kind="ExternalInput")
    wqc_d = nc.dram_tensor("wqc", [P, NT, C], f8, kind="ExternalInput")
    wvt_d = nc.dram_tensor("wvt", [P, NT, C], f8, kind="ExternalInput")
    # gn affine columns: [:, 0, :]=gamma, [:, 1, :]=beta
    gab_d = nc.dram_tensor("gn_ab", [P, 2, NT], f32, kind="ExternalInput")
    # block-diagonal group-average matrix: GG[k,p] = 1/16 iff k//16 == p//16
    gg_d = nc.dram_tensor("gg", [P, P], f32, kind="ExternalInput")
    if not zero_out_bias:
        btp_d = nc.dram_tensor("btp", [P, NT], f32, kind="ExternalInput")
    out_d = nc.dram_tensor("out", [NB, C, HW], f32, kind="ExternalOutput")

    with tile.TileContext(nc) as tc:
        with (
            tc.tile_pool(name="consts", bufs=1) as consts,
            tc.tile_pool(name="hp", bufs=2) as hp,
            tc.tile_pool(name="qcp", bufs=2) as qcp,
            tc.tile_pool(name="vtp", bufs=2) as vtp,
            tc.tile_pool(name="est", bufs=1) as estp,
            tc.tile_pool(name="sump", bufs=1) as sump,
            tc.tile_pool(name="repp", bufs=1) as repp,
            # tmpp=4: with only 2 bufs, tmp(cm3)'s alloc waits the GpSimd
            # residual-add of cm1, putting the slow GpSimd chain on the
            # critical path into scores(b+1) via the psb rotation (measured
            # ~5us/iter stall).
            tc.tile_pool(name="tmpp", bufs=4) as tmpp,
            tc.tile_pool(name="outp", bufs=4) as outp,
            tc.tile_pool(name="small", bufs=2) as small,
            # psb 2x[P,1024] (4) + psq 2x[P,512] (2) + rsp 2x[P,512] (2)
            # = all 8 banks. The den-reduce psums get their OWN pool (rsp):
            # inside the psq rotation, qc(b+1)'s first psum alloc waits on
            # recip(b), serializing the whole next-sample projection phase
            # behind the exp->gpsimd-tree->den-red->recip latency chain
            # (~10us/iter, measured).
            tc.tile_pool(name="psb", bufs=2, space="PSUM") as psb,
            tc.tile_pool(name="psq", bufs=2, space="PSUM") as psq,
            tc.tile_pool(name="rsp", bufs=2, space="PSUM") as rsp,
        ):
            x_sb = consts.tile([P, NB, NT, HW], f32, tag="x")
            wqc_sb = consts.tile([P, NT, C], f8, tag="wqc")
            wvt_sb = consts.tile([P, NT, C], f8, tag="wvt")
            gab_sb = consts.tile([P, 2, NT], f32, tag="gab")
            gg_sb = consts.tile([P, P], f32, tag="gg")
            if not zero_out_bias:
                btp_sb = consts.tile([P, NT], f32, tag="btp")

            # gg+gab first on gpsimd (tiny, needed by affine(0)); then x[0]
            # halves (2KB contiguous per partition row) own ALL THREE rings.
            # DMA engines pull from all queued transfers CONCURRENTLY (not
            # ring-order), so x[1..3] loads are semaphore-GATED behind GN
            # progress below -- otherwise they steal ~2/3 of the wire and
            # x[0] (which gates the whole GN(0)->qc(0) prologue chain) lands
            # ~7us late (measured).
            # Each DMA transfer is serviced by ~one engine at ~21 B/ns, so
            # wire parallelism = in-flight transfer count. Transfers must
            # keep 2KB-contiguous rows and FULL 128 partitions (64-partition
            # transfers run at half rate -- measured), so a 2MB sample maxes
            # out at 8 transfers = ~12us. x[1..3] are semaphore-gated behind
            # GN progress so they don't steal the wire from x[0].
            nc.gpsimd.dma_start(out=gg_sb[:, :], in_=gg_d[:, :])
            nc.gpsimd.dma_start(out=gab_sb[:, :, :], in_=gab_d[:, :, :])
            engs = (nc.sync, nc.scalar, nc.gpsimd)
            qi = 0
            for t in range(NT):
                for h0 in (0, 512):
                    engs[qi % 3].dma_start(
                        out=x_sb[:, 0, t, h0:h0 + 512],
                        in_=x_d[0, t * P:(t + 1) * P, h0:h0 + 512])
                    qi += 1
            # weights in 2 transfers each, right behind x[0]
            for t2 in (0, 2):
                nc.sync.dma_start(out=wqc_sb[:, t2:t2 + 2, :],
                                  in_=wqc_d[:, t2:t2 + 2, :])
                nc.scalar.dma_start(out=wvt_sb[:, t2:t2 + 2, :],
                                    in_=wvt_d[:, t2:t2 + 2, :])
            if not zero_out_bias:
                nc.gpsimd.dma_start(out=btp_sb[:, :], in_=btp_d[:, :])

            def load_x(b, eng_list, after):
                """Issue sample b's x as 8 half-tile transfers (full rate)
                round-robined over eng_list, gated behind `after`."""
                qi = 0
                for t in range(NT):
                    for h0 in (0, 512):
                        eng = eng_list[qi % len(eng_list)]
                        qi += 1
                        dd = eng.dma_start(
                            out=x_sb[:, b, t, h0:h0 + 512],
                            in_=x_d[b, t * P:(t + 1) * P, h0:h0 + 512])
                        if after is not None:
                            tile.add_dep_helper(dd.ins, after.ins, sync=True,
                                                reason="x load gated on GN progress")

            # 8.0-matrix: reduces den partials across partitions AND folds the
            # 8x fused-weight scale (rep = 1/(8*den))
            ones_sb = consts.tile([P, P], bf16, tag="ones")
            nc.vector.memset(ones_sb[:, :], WSC)
            magic_sb = consts.tile([P, NT], mybir.dt.int32, tag="magic")
            nc.vector.memset(magic_sb[:, :], 0x5F3759DF)
            # per-partition exp-offset column (bias operand for the Exp calls)
            eoff_sb = consts.tile([P, 1], f32, tag="eoff")
            nc.vector.memset(eoff_sb[:, :], EOFF)
            # dummy Exp: pulls the ACT Exp-table load into the idle prologue
            expwarm = small.tile([P, 1], f32, tag="expwarm")
            nc.scalar.activation(out=expwarm[:, :], in_=ones_sb[:, 0:1],
                                 func=ACT.Exp, scale=CINV / WSC,
                                 bias=eoff_sb[:, 0:1])

            a_all = consts.tile([P, NB, NT], f32, tag="a_all")
            b_all = consts.tile([P, NB, NT], f32, tag="b_all")

            # PE warm-up on gg (earliest-arriving tensor): releases the HAM
            # clock-gate before the real stream.
            warm_ps = psq.tile([P, 512], f32, tag="qkv")
            for w in range(5):
                nc.tensor.matmul(
                    warm_ps[:, 0:128], gg_sb[:, 0:128], gg_sb[:, 0:128],
                    start=(w == 0), stop=(w == 4),
                )

            def gn_stats(b, after=None):
                """bn stats -> per-channel (mean, Ex2) packed in mv."""
                mv = small.tile([P, NT, 2], f32, tag="mv")
                nsub = 2
                step = HW // nsub
                first = None
                for t in range(NT):
                    st6 = small.tile([P, nsub, 6], f32, tag="st6")
                    for q in range(nsub):
                        iq = nc.vector.bn_stats(
                            out=st6[:, q, :],
                            in_=x_sb[:, b, t, q * step:(q + 1) * step])
                        if first is None:
                            first = iq
                        if after is not None:
                            tile.add_dep_helper(iq.ins, after.ins, sync=False,
                                                reason="gn stats after prev apply")
                    nc.vector.bn_aggr(out=mv[:, t, :], in_=st6[:, :, :])
                msq = small.tile([P, NT], f32, tag="msq")
                nc.vector.tensor_mul(msq[:, :], mv[:, :, 0], mv[:, :, 0])
                nc.vector.tensor_add(mv[:, :, 1], mv[:, :, 1], msq[:, :])
                return mv, first

            def gn_affine(b, mv):
                """fused group-avg+broadcast matmul, then per-channel A/B."""
                bc_ps = psq.tile([P, 512], f32, tag="qkv")
                nc.tensor.matmul(bc_ps[:, :NT * 2], gg_sb[:, :], mv[:, :, :],
                                 start=True, stop=True)
                bc = small.tile([P, NT, 2], f32, tag="bcs")
                nc.vector.tensor_copy(bc[:, :, :], bc_ps[:, 0:NT * 2])
                vb = small.tile([P, NT], f32, tag="vb")
                nc.vector.tensor_mul(vb[:, :], bc[:, :, 0], bc[:, :, 0])
                nc.vector.tensor_sub(vb[:, :], bc[:, :, 1], vb[:, :])
                nc.vector.tensor_scalar_add(vb[:, :], vb[:, :], EPS)
                # rstd = rsqrt(var+eps): fast-inverse-sqrt + 1 Newton step
                # (all-DVE: keeps Sqrt off ACT so it never evicts Exp)
                ii = small.tile([P, NT], mybir.dt.int32, tag="ii")
                nc.vector.tensor_scalar(
                    out=ii[:, :], in0=vb.bitcast(mybir.dt.int32)[:, :],
                    scalar1=1, scalar2=None, op0=ALU.arith_shift_right)
                nc.vector.tensor_tensor(ii[:, :], magic_sb[:, :], ii[:, :],
                                        op=ALU.subtract)
                y0 = ii.bitcast(f32)
                yt = small.tile([P, NT], f32, tag="yt")
                y1 = small.tile([P, NT], f32, tag="y1")
                nc.vector.tensor_mul(yt[:, :], vb[:, :], y0[:, :])
                nc.vector.tensor_mul(yt[:, :], yt[:, :], y0[:, :])
                nc.vector.tensor_scalar(out=yt[:, :], in0=yt[:, :], scalar1=-0.5,
                                        scalar2=1.5, op0=ALU.mult, op1=ALU.add)
                nc.vector.tensor_mul(y1[:, :], y0[:, :], yt[:, :])
                tmp = small.tile([P, NT], f32, tag="tmpab")
                nc.vector.tensor_mul(a_all[:, b, :], y1[:, :], gab_sb[:, 0, :])
                nc.vector.tensor_mul(tmp[:, :], bc[:, :, 0], a_all[:, b, :])
                nc.vector.tensor_sub(b_all[:, b, :], gab_sb[:, 1, :], tmp[:, :])

            def apply_h(b, after=None):
                """h = x*A + B (fp8), all-DVE. (NOT on ACT: the Activation
                engine has exec-queue depth 0, so an apply waiting on the
                DVE affine chain head-blocks the whole exp stream -- cost
                ~5us/iter, measured. DVE's depth-8 queue absorbs it.)"""
                h = hp.tile([P, NT, HW], f8, tag="h")
                last = None
                for t in range(NT):
                    last = nc.vector.tensor_scalar(
                        out=h[:, t, :], in0=x_sb[:, b, t, :],
                        scalar1=a_all[:, b, t:t + 1],
                        scalar2=b_all[:, b, t:t + 1],
                        op0=ALU.mult, op1=ALU.add,
                    )
                    if after is not None:
                        tile.add_dep_helper(last.ins, after.ins, sync=False,
                                            reason="apply after prev tmp")
                return h, last

            def qc_proj(b, h_sb):
                """qc = A^T h (fp8 [d, hw], 8x-scaled); psum->sbuf on ACT."""
                qc_sb = qcp.tile([P, NT, HW], f8, tag="qc")
                for dm in range(NT):
                    ps0 = psq.tile([P, 512], f32, tag="qkv")
                    ps1 = psq.tile([P, 512], f32, tag="qkv")
                    for kp in range(NP):
                        lhsT = wqc_sb[:, 2 * kp:2 * kp + 2, dm * P:(dm + 1) * P]
                        st, sp = (kp == 0), (kp == NP - 1)
                        nc.tensor.matmul(ps0[:, :], lhsT,
                                         h_sb[:, 2 * kp:2 * kp + 2, 0:512],
                                         start=st, stop=sp, perf_mode=DR)
                        nc.tensor.matmul(ps1[:, :], lhsT,
                                         h_sb[:, 2 * kp:2 * kp + 2, 512:1024],
                                         start=st, stop=sp, perf_mode=DR)
                    nc.scalar.copy(qc_sb[:, dm, 0:512], ps0[:, :])
                    nc.scalar.copy(qc_sb[:, dm, 512:1024], ps1[:, :])
                return qc_sb

            def vt_proj(b, h_sb, split_copies=False):
                """vt' = (Wv Wt)^T h, transposed [hw, d] fp8 (8x-scaled);
                psum->sbuf on ACT (alternating with DVE for the deferred
                vt'(0), whose copies would otherwise stack up behind qc(1)'s
                in iteration 0's ACT queue)."""
                vt_sb = vtp.tile([P, NJ, C], f8, tag="vt")
                for jm in range(NJ):
                    ps = psq.tile([P, 512], f32, tag="qkv")
                    for kp in range(NP):
                        nc.tensor.matmul(
                            ps[:, :],
                            h_sb[:, 2 * kp:2 * kp + 2, jm * P:(jm + 1) * P],
                            wvt_sb[:, 2 * kp:2 * kp + 2, :],
                            start=(kp == 0), stop=(kp == NP - 1), perf_mode=DR,
                        )
                    if split_copies and jm % 2 == 0:
                        nc.vector.tensor_copy(vt_sb[:, jm, :], ps[:, :])
                    else:
                        nc.scalar.copy(vt_sb[:, jm, :], ps[:, :])
                return vt_sb

            # ---------------- prologue ----------------
            # vt'(0) is NOT issued here: it would delay scores(0) by 3.4us of
            # PE time; it slots into iteration 0 between qc(1) and vt'(1),
            # where its ACT copies run after exp(0) drains.
            mv0, stats0_first = gn_stats(0)
            # x[1] loads fire once GN(0) is underway (x[0] fully landed soon
            # after); x[2]/x[3] once GN(1) starts: keeps the wire exclusive
            # to whatever the pipeline needs next.
            load_x(1, (nc.sync, nc.scalar), stats0_first)
            gn_affine(0, mv0)
            h0_sb, last_apply = apply_h(0)

            qc_next = qc_proj(0, h0_sb)

            mv1, stats1_first = gn_stats(1, after=last_apply)
            load_x(2, (nc.gpsimd, nc.sync), stats1_first)
            load_x(3, (nc.sync, nc.gpsimd), stats1_first)
            gn_affine(1, mv1)
            h_next, last_apply = apply_h(1)

            vt_next = None
            h_cur = h0_sb

            # ---------------- main loop ----------------
            mv_next = None
            for b in range(NB):
                qc_sb = qc_next
                vt_sb = vt_next
                h_sb = h_next

                # iter>=1: GN stats for b+2 at the DVE-queue HEAD -- its x
                # landed long ago, it has no cross-engine deps, and it fills
                # the DVE idle while exp(b) produces the den-tree inputs.
                # (iter 0 slots stats(2) after recip(0) instead: x[2] may
                # still be in flight at iteration-0 start.)
                gn_head = b >= 1 and b + 2 < NB
                if gn_head:
                    mv_next, _ = gn_stats(b + 2, after=last_apply)

                # ---- scores transposed (ST[j,i] = sum_d h[d,j] qc[d,i])
                #      + exp (ACT, fp8 out with -2 offset) ----
                est_sb = estp.tile([P, NJ, HW], f8e5, tag="est")
                for jm in range(NJ):
                    st_ps = psb.tile([P, HW], f32, tag="big")
                    for kp in range(NP):
                        lhsT = h_cur[:, 2 * kp:2 * kp + 2, jm * P:(jm + 1) * P]
                        st, sp = (kp == 0), (kp == NP - 1)
                        nc.tensor.matmul(st_ps[:, 0:512], lhsT,
                                         qc_sb[:, 2 * kp:2 * kp + 2, 0:512],
                                         start=st, stop=sp, perf_mode=DR)
                        nc.tensor.matmul(st_ps[:, 512:1024], lhsT,
                                         qc_sb[:, 2 * kp:2 * kp + 2, 512:1024],
                                         start=st, stop=sp, perf_mode=DR)
                    nc.scalar.activation(
                        out=est_sb[:, jm, :], in_=st_ps[:, :], func=ACT.Exp,
                        scale=CINV / WSC, bias=eoff_sb[:, 0:1],
                    )

                # ---- softmax denominator tree (bf16 out): level-1 split
                # 2 adds DVE / 2 adds GpSimd (idle early-iteration) ----
                s0 = sump.tile([P, HW], bf16, tag="s0")
                s1 = sump.tile([P, HW], bf16, tag="s1")
                s2 = sump.tile([P, HW], bf16, tag="s2")
                s3 = sump.tile([P, HW], bf16, tag="s3")
                nc.vector.tensor_add(s0[:, :], est_sb[:, 0, :], est_sb[:, 1, :])
                nc.gpsimd.tensor_add(s1[:, :], est_sb[:, 2, :], est_sb[:, 3, :])
                nc.vector.tensor_add(s2[:, :], est_sb[:, 4, :], est_sb[:, 5, :])
                nc.gpsimd.tensor_add(s3[:, :], est_sb[:, 6, :], est_sb[:, 7, :])
                nc.vector.tensor_add(s0[:, :], s0[:, :], s1[:, :])
                nc.vector.tensor_add(s2[:, :], s2[:, :], s3[:, :])
                nc.vector.tensor_add(s0[:, :], s0[:, :], s2[:, :])

                # head-GN completes here: the affine bc matmul lands in the
                # PE queue right after scores(b) (runs ~+7), so apply(b+2)
                # finishes ~+11 -- well before qc(b+2) needs h(b+2) at +6.8
                # of the NEXT iteration. (At the tail it finished ~+9 into
                # the next iteration and stalled qc(b+2) ~5us, measured.)
                if gn_head:
                    gn_affine(b + 2, mv_next)
                    h_nn, last_apply = apply_h(b + 2, after=last_tmp)

                # ---- qc, vt' for sample b+1 fill TensorE while ACT exps ----
                if b + 1 < NB:
                    qc_next = qc_proj(b + 1, h_next)
                if b == 0:
                    vt_sb = vt_proj(0, h0_sb, split_copies=True)
                if b + 1 < NB:
                    vt_next = vt_proj(b + 1, h_next)

                # ---- PV fp8 DoubleRow over jm pairs: psum IS the final
                #      pre-residual activation (out-proj fused into vt').
                #      The den partition-reduce slots in AFTER PV cm0: rep is
                #      first needed by tmp(cm0), so this buys the DVE den
                #      tree ~2.3us of extra slack before the PE blocks on it.
                last_tmp = None
                rep = repp.tile([P, HW], f32, tag="rep")
                for cm in range(NT):
                    o_ps = psb.tile([P, HW], f32, tag="big")
                    for t in range(NJP):
                        lhsT = vt_sb[:, 2 * t:2 * t + 2, cm * P:(cm + 1) * P]
                        st, sp = (t == 0), (t == NJP - 1)
                        nc.tensor.matmul(o_ps[:, 0:512], lhsT,
                                         est_sb[:, 2 * t:2 * t + 2, 0:512],
                                         start=st, stop=sp, perf_mode=DR)
                        nc.tensor.matmul(o_ps[:, 512:1024], lhsT,
                                         est_sb[:, 2 * t:2 * t + 2, 512:1024],
                                         start=st, stop=sp, perf_mode=DR)
                    if cm == 0:
                        rs0 = rsp.tile([P, 512], f32, tag="rs")
                        rs1 = rsp.tile([P, 512], f32, tag="rs")
                        nc.tensor.matmul(rs0[:, :], ones_sb[:, :], s0[:, 0:512],
                                         start=True, stop=True)
                        nc.tensor.matmul(rs1[:, :], ones_sb[:, :],
                                         s0[:, 512:1024], start=True, stop=True)
                        nc.vector.reciprocal_approx_fast(out=rep[:, 0:512],
                                                         in_=rs0[:, :])
                        nc.vector.reciprocal_approx_fast(out=rep[:, 512:1024],
                                                         in_=rs1[:, :])
                        if b == 0 and b + 2 < NB:
                            # iter-0 stats(2): after recip (x[2] landed by
                            # now), before the tmps -- fills DVE idle without
                            # delaying the scores(1) psb chain
                            mv_next, _ = gn_stats(b + 2, after=last_apply)
                    # tmp = psum * rep on DVE; residual add on GpSimd
                    # (DVE for the last sample: shorter tail); DMA issues on
                    # the idle Sync engine (~600ns engine time each)
                    tmv = tmpp.tile([P, HW], f32, tag="tmp")
                    last_tmp = nc.vector.tensor_mul(tmv[:, :], o_ps[:, :],
                                                    rep[:, :])
                    out_t = outp.tile([P, HW], f32, tag="out")
                    for hh in (0, 512):
                        if zero_out_bias:
                            eng = nc.vector if b == NB - 1 else nc.gpsimd
                            eng.tensor_add(out_t[:, hh:hh + 512],
                                           tmv[:, hh:hh + 512],
                                           x_sb[:, b, cm, hh:hh + 512])
                        else:
                            nc.vector.scalar_tensor_tensor(
                                out=out_t[:, hh:hh + 512],
                                in0=tmv[:, hh:hh + 512],
                                scalar=btp_sb[:, cm:cm + 1],
                                in1=x_sb[:, b, cm, hh:hh + 512],
                                op0=ALU.add, op1=ALU.add,
                            )
                        deng = nc.scalar if (b == NB - 1 and hh) else nc.sync
                        deng.dma_start(
                            out=out_d[b, cm * P:(cm + 1) * P, hh:hh + 512],
                            in_=out_t[:, hh:hh + 512])

                # h rotation; iter-0's affine+apply(2) at the tail (its bc
                # matmul runs after PV(0), apply lands well before iter-1's
                # qc(2)); head-GN iters already produced h_nn above.
                if b + 2 < NB:
                    h_cur = h_next
                    if gn_head:
                        h_next = h_nn
                    else:
                        gn_affine(b + 2, mv_next)
                        h_next, last_apply = apply_h(b + 2, after=last_tmp)
                else:
                    h_cur = h_next

    nc.compile()
    return nc


def prep_inputs(inputs):
    """Host-side prep: per-core in_maps with fused, pre-laid-out weights."""
    e4 = ml_dtypes.float8_e4m3
    x = np.ascontiguousarray(np.asarray(inputs["x"], dtype=np.float32)).reshape(
        B_FULL, C, HW
    )

    Wq = np.asarray(inputs["Wq"], dtype=np.float64)
    Wk = np.asarray(inputs["Wk"], dtype=np.float64)
    Wv = np.asarray(inputs["Wv"], dtype=np.float64)
    Wt = np.asarray(inputs["Wt"], dtype=np.float64)
    A = (WSC * (Wq @ Wk.T)).astype(np.float32)   # scores = h^T A h / 8
    Bm = (WSC * (Wv @ Wt)).astype(np.float32)    # out = (B^T h)^T P / 8

    def wprep(w):
        # [C, C] -> [P, NT, C]  (lhsT slices w[kc*128+p, d])
        return np.ascontiguousarray(
            np.asarray(w, dtype=np.float32).reshape(NT, P, C).transpose(1, 0, 2)
        ).astype(e4)

    def cols(v):
        # [C] -> [P, NT]
        return np.ascontiguousarray(
            np.asarray(v, dtype=np.float32).reshape(NT, P).T
        )

    gab = np.stack([cols(inputs["gn_scale"]), cols(inputs["gn_bias"])], axis=1)
    gg = np.zeros((P, P), np.float32)
    for p in range(P):
        gg[p, (p // GS) * GS:(p // GS + 1) * GS] = 1.0 / GS

    btp = (np.asarray(inputs["bt"], dtype=np.float64)
           + Wt.T @ np.asarray(inputs["bv"], dtype=np.float64)).astype(np.float32)

    shared = {
        "wqc": wprep(A), "wvt": wprep(Bm),
        "gn_ab": np.ascontiguousarray(gab), "gg": gg,
    }
    if np.any(btp != 0):
        shared["btp"] = cols(btp)
    in_maps = []
    for c_id in range(N_CORES):
        m = dict(shared)
        m["x"] = np.ascontiguousarray(x[c_id * NB:(c_id + 1) * NB])
        in_maps.append(m)
    return in_maps


_NC_CACHE = {}


def get_nc(zero_out_bias=True):
    key = (zero_out_bias,)
    if key not in _NC_CACHE:
        _NC_CACHE[key] = build_nc(zero_out_bias=zero_out_bias)
    return _NC_CACHE[key]


def _kernel_numpy(inputs):
    """Pure-numpy fallback, only for nonzero bq/bk (never hit by this
    problem's inputs -- setup_inputs() uses zero biases)."""
    x = np.asarray(inputs["x"], dtype=np.float64)
    B, C_, H_, W_ = x.shape
    g = x.reshape(B, NGROUPS, C_ // NGROUPS, H_, W_)
    mean = g.mean(axis=(2, 3, 4), keepdims=True)
    var = g.var(axis=(2, 3, 4), keepdims=True)
    hn = ((g - mean) / np.sqrt(var + EPS)).reshape(B, C_, H_, W_)
    hn = hn * np.asarray(inputs["gn_scale"], np.float64)[None, :, None, None] \
        + np.asarray(inputs["gn_bias"], np.float64)[None, :, None, None]

    def nin(h, Wm, bm):
        return np.einsum("bchw,cd->bdhw", h, np.asarray(Wm, np.float64)) \
            + np.asarray(bm, np.float64)[None, :, None, None]

    q = nin(hn, inputs["Wq"], inputs["bq"])
    k = nin(hn, inputs["Wk"], inputs["bk"])
    v = nin(hn, inputs["Wv"], inputs["bv"])
    w = np.einsum("bchw,bcij->bhwij", q, k) * (C_ ** -0.5)
    w = w.reshape(B, H_, W_, H_ * W_)
    w = np.exp(w - w.max(axis=-1, keepdims=True))
    w = (w / w.sum(axis=-1, keepdims=True)).reshape(B, H_, W_, H_, W_)
    hh = np.einsum("bhwij,bcij->bchw", w, v)
    hh = nin(hh, inputs["Wt"], inputs["bt"])
    return (hh + x).astype(np.float32)


def run(inputs, trace=False):
    from concourse.bass_utils import run_bass_kernel_spmd

    if not (np.all(np.asarray(inputs["bq"]) == 0)
            and np.all(np.asarray(inputs["bk"]) == 0)):
        return _kernel_numpy(inputs), None

    in_maps = prep_inputs(inputs)
    nc = get_nc(zero_out_bias="btp" not in in_maps[0])
    res = run_bass_kernel_spmd(
        nc, in_maps, core_ids=list(range(N_CORES)), trace=trace
    )
    out = np.concatenate([np.asarray(r["out"]) for r in res.results], axis=0)
    return out.reshape(B_FULL, C, H, W), res


def kernel(**inputs):
    out, _ = run(inputs, trace=False)
    return out


# revision 41
# speedup vs baseline: 1.0931x; 1.0931x over previous
"""AttnBlock (GroupNorm -> QKV 1x1 -> full attention over 1024 tokens -> out-proj
+ residual) for x [32, 512, 32, 32] f32, distributed data-parallel over 8
NeuronCores (4 samples per core, weights replicated).

Per-core single-NC Bass/Tile kernel, v2: weight-fusion + all-fp8 DoubleRow.

Algebraic restructuring (host-side, exact in f32):
  - scores  = (Wq h)^T (Wk h) = h^T A h with A = Wq @ Wk.T  -> ONE fused
    "qc" projection (qc = A^T h) replaces the separate Q and K projections.
  - out-proj fused into V: vt' = (Wv @ Wt)^T h gives
    out = vt'^T P_norm  directly, so the PV matmul's PSUM already holds the
    final pre-residual activation -- the separate out-projection disappears.
  - A and B=Wv@Wt are pre-scaled by 8 on the host so their entries clear the
    fp8e4 subnormal range; the 8x cancels via exp-scale (CINV/8) on the
    score side and via rep = 1/(8*den) on the PV side.

Per-sample PE work drops 82944 -> ~50200 col-cycles (scores 16384 + PV-DR
16384 + qc 8192 + vt' 8192 + den 1024): every matmul issues moving columns at
1/cycle regardless of dtype, so fp8 DoubleRow's 2x comes purely from halving
the pass count (contraction 256 rows/pass).

  - est = exp(s*c - 4.5) stored fp8e5 (e5m2: scores empirically reach 13.8
    sigma, far past fp8e4's e^11.7 dynamic range; e5m2 spans it easily and
    softmax normalization cancels most of its 2-mantissa-bit noise -- host
    emulation: 0.0078 rel err vs 0.0068 for bf16 est). The offset cancels
    exactly in softmax. PV runs fp8 DoubleRow (e4m3 vt x e5m2 est) over
    jm-pair passes.
  - softmax denominator: DVE pairwise-add tree over the 8 fp8 exp tiles
    (bf16 out, 2x DVE rate), one 8.0-matrix bf16 matmul reduces partitions
    and replicates 8*den; rep = 1/(8 den) via reciprocal_approx_fast.
  - residual: tmp = PV_psum * rep on DVE, out = tmp + x on GpSimd (idle
    otherwise; DVE for the last sample to shorten the tail), DMA per half.
  - engine balance per sample: PE ~20.9us, DVE ~19.8us (GN stats/apply, den
    tree, recip, tmp), ACT ~17.4us (exp + qc/vt' psum->sbuf fp8 copies),
    GpSimd ~9us (residual adds + out DMA issue).
  - prologue: x[0] owns all three DMA rings before weights/x[1..3] queue up;
    GN(0) stats start per-tile as x[0] tiles land.
"""

import os
import sys

import numpy as np

sys.path.insert(0, "/opt/trn_rl_repo")

import ml_dtypes  # noqa: E402

import concourse.bass as bass  # noqa: E402
import concourse.tile as tile  # noqa: E402
from concourse import bacc, mybir  # noqa: E402

P = 128
B_FULL, C, H, W = 32, 512, 32, 32
HW = H * W            # 1024 tokens
N_CORES = 8
NB = B_FULL // N_CORES  # 4 samples per core
NT = C // P           # 4 channel tiles
NP = NT // 2          # 2 DoubleRow channel-tile pairs
NJ = HW // P          # 8 token tiles
NJP = NJ // 2         # 4 DoubleRow token-tile pairs
NGROUPS = 32
GS = C // NGROUPS     # 16 channels per group
EPS = 1e-6
CINV = float(C) ** -0.5
WSC = 8.0             # fused weights pre-scaled by 8 (fp8 subnormal guard)
EOFF = -4.5           # exp offset: est = exp(s*c - 4.5), cancels in softmax

f32 = mybir.dt.float32
bf16 = mybir.dt.bfloat16
f8 = mybir.dt.float8e4
f8e5 = mybir.dt.float8e5
ALU = mybir.AluOpType
ACT = mybir.ActivationFunctionType
DR = mybir.MatmulPerfMode.DoubleRow


def build_nc(zero_out_bias=True):
    """Build the single-core Bass graph (SPMD: same graph on all 8 cores).

    zero_out_bias: fused output bias bt' = bt + Wt^T bv is all-zero (true for
    this problem), so the residual add drops the bias column.
    """
    nc = bacc.Bacc("TRN2", target_bir_lowering=False, debug=False)

    x_d = nc.dram_tensor("x", [NB, C, HW], f32, kind="ExternalInput")
    wqc_d = nc.dram_tensor("wqc", [P, NT, C], f8, kind="ExternalInput")
    wvt_d = nc.dram_tensor("wvt", [P, NT, C], f8, kind="ExternalInput")
    # gn affine columns: [:, 0, :]=gamma, [:, 1, :]=beta
    gab_d = nc.dram_tensor("gn_ab", [P, 2, NT], f32, kind="ExternalInput")
    # block-diagonal group-average matrix: GG[k,p] = 1/16 iff k//16 == p//16
    gg_d = nc.dram_tensor("gg", [P, P], f32, kind="ExternalInput")
    if not zero_out_bias:
        btp_d = nc.dram_tensor("btp", [P, NT], f32, kind="ExternalInput")
    out_d = nc.dram_tensor("out", [NB, C, HW], f32, kind="ExternalOutput")

    with tile.TileContext(nc) as tc:
        with (
            tc.tile_pool(name="consts", bufs=1) as consts,
            tc.tile_pool(name="hp", bufs=2) as hp,
            tc.tile_pool(name="qcp", bufs=2) as qcp,
            tc.tile_pool(name="vtp", bufs=2) as vtp,
            tc.tile_pool(name="est", bufs=1) as estp,
            tc.tile_pool(name="sump", bufs=1) as sump,
            tc.tile_pool(name="repp", bufs=1) as repp,
            # tmpp=4: with only 2 bufs, tmp(cm3)'s alloc waits the GpSimd
            # residual-add of cm1, putting the slow GpSimd chain on the
            # critical path into scores(b+1) via the psb rotation (measured
            # ~5us/iter stall).
            tc.tile_pool(name="tmpp", bufs=4) as tmpp,
            tc.tile_pool(name="outp", bufs=4) as outp,
            tc.tile_pool(name="small", bufs=2) as small,
            # psb 2x[P,1024] (4) + psq 2x[P,512] (2) + rsp 2x[P,512] (2)
            # = all 8 banks. The den-reduce psums get their OWN pool (rsp):
            # inside the psq rotation, qc(b+1)'s first psum alloc waits on
            # recip(b), serializing the whole next-sample projection phase
            # behind the exp->gpsimd-tree->den-red->recip latency chain
            # (~10us/iter, measured).
            tc.tile_pool(name="psb", bufs=2, space="PSUM") as psb,
            tc.tile_pool(name="psq", bufs=2, space="PSUM") as psq,
            tc.tile_pool(name="rsp", bufs=2, space="PSUM") as rsp,
        ):
            x_sb = consts.tile([P, NB, NT, HW], f32, tag="x")
            wqc_sb = consts.tile([P, NT, C], f8, tag="wqc")
            wvt_sb = consts.tile([P, NT, C], f8, tag="wvt")
            gab_sb = consts.tile([P, 2, NT], f32, tag="gab")
            gg_sb = consts.tile([P, P], f32, tag="gg")
            if not zero_out_bias:
                btp_sb = consts.tile([P, NT], f32, tag="btp")

            # gg+gab first on gpsimd (tiny, needed by affine(0)); then x[0]
            # halves (2KB contiguous per partition row) own ALL THREE rings.
            # DMA engines pull from all queued transfers CONCURRENTLY (not
            # ring-order), so x[1..3] loads are semaphore-GATED behind GN
            # progress below -- otherwise they steal ~2/3 of the wire and
            # x[0] (which gates the whole GN(0)->qc(0) prologue chain) lands
            # ~7us late (measured).
            # Each DMA transfer is serviced by ~one engine at ~21 B/ns, so
            # wire parallelism = in-flight transfer count. Transfers must
            # keep 2KB-contiguous rows and FULL 128 partitions (64-partition
            # transfers run at half rate -- measured), so a 2MB sample maxes
            # out at 8 transfers = ~12us. x[1..3] are semaphore-gated behind
            # GN progress so they don't steal the wire from x[0].
            nc.gpsimd.dma_start(out=gg_sb[:, :], in_=gg_d[:, :])
            nc.gpsimd.dma_start(out=gab_sb[:, :, :], in_=gab_d[:, :, :])
            engs = (nc.sync, nc.scalar, nc.gpsimd)
            qi = 0
            for t in range(NT):
                for h0 in (0, 512):
                    engs[qi % 3].dma_start(
                        out=x_sb[:, 0, t, h0:h0 + 512],
                        in_=x_d[0, t * P:(t + 1) * P, h0:h0 + 512])
                    qi += 1
            # weights in 2 transfers each, right behind x[0]
            for t2 in (0, 2):
                nc.sync.dma_start(out=wqc_sb[:, t2:t2 + 2, :],
                                  in_=wqc_d[:, t2:t2 + 2, :])
                nc.scalar.dma_start(out=wvt_sb[:, t2:t2 + 2, :],
                                    in_=wvt_d[:, t2:t2 + 2, :])
            if not zero_out_bias:
                nc.gpsimd.dma_start(out=btp_sb[:, :], in_=btp_d[:, :])

            def load_x(b, eng_list, after):
                """Issue sample b's x as 8 half-tile transfers (full rate)
                round-robined over eng_list, gated behind `after`."""
                qi = 0
                for t in range(NT):
                    for h0 in (0, 512):
                        eng = eng_list[qi % len(eng_list)]
                        qi += 1
                        dd = eng.dma_start(
                            out=x_sb[:, b, t, h0:h0 + 512],
                            in_=x_d[b, t * P:(t + 1) * P, h0:h0 + 512])
                        if after is not None:
                            tile.add_dep_helper(dd.ins, after.ins, sync=True,
                                                reason="x load gated on GN progress")

            # 8.0-matrix: reduces den partials across partitions AND folds the
            # 8x fused-weight scale (rep = 1/(8*den))
            ones_sb = consts.tile([P, P], bf16, tag="ones")
            nc.vector.memset(ones_sb[:, :], WSC)
            magic_sb = consts.tile([P, NT], mybir.dt.int32, tag="magic")
            nc.vector.memset(magic_sb[:, :], 0x5F3759DF)
            # per-partition exp-offset column (bias operand for the Exp calls)
            eoff_sb = consts.tile([P, 1], f32, tag="eoff")
            nc.vector.memset(eoff_sb[:, :], EOFF)
            # dummy Exp: pulls the ACT Exp-table load into the idle prologue
            expwarm = small.tile([P, 1], f32, tag="expwarm")
            nc.scalar.activation(out=expwarm[:, :], in_=ones_sb[:, 0:1],
                                 func=ACT.Exp, scale=CINV / WSC,
                                 bias=eoff_sb[:, 0:1])

            a_all = consts.tile([P, NB, NT], f32, tag="a_all")
            b_all = consts.tile([P, NB, NT], f32, tag="b_all")

            # PE warm-up on gg (earliest-arriving tensor): releases the HAM
            # clock-gate before the real stream.
            warm_ps = psq.tile([P, 512], f32, tag="qkv")
            for w in range(5):
                nc.tensor.matmul(
                    warm_ps[:, 0:128], gg_sb[:, 0:128], gg_sb[:, 0:128],
                    start=(w == 0), stop=(w == 4),
                )

            def gn_stats(b, after=None, mv=None, ts=0, te=NT):
                """bn stats -> per-channel (mean, Ex2) in mv[:, ts:te, :].
                GN groups (16ch) never span 128-channel tiles, so the whole
                GN chain is per-tile independent -- the prologue exploits
                this with 2-tile batches that chase the x DMAs."""
                if mv is None:
                    mv = small.tile([P, NT, 2], f32, tag="mv")
                nsub = 2
                step = HW // nsub
                first = None
                for t in range(ts, te):
                    st6 = small.tile([P, nsub, 6], f32, tag="st6")
                    for q in range(nsub):
                        iq = nc.vector.bn_stats(
                            out=st6[:, q, :],
                            in_=x_sb[:, b, t, q * step:(q + 1) * step])
                        if first is None:
                            first = iq
                        if after is not None:
                            tile.add_dep_helper(iq.ins, after.ins, sync=False,
                                                reason="gn stats after prev apply")
                    nc.vector.bn_aggr(out=mv[:, t, :], in_=st6[:, :, :])
                msq = small.tile([P, NT], f32, tag="msq")
                nc.vector.tensor_mul(msq[:, ts:te], mv[:, ts:te, 0],
                                     mv[:, ts:te, 0])
                nc.vector.tensor_add(mv[:, ts:te, 1], mv[:, ts:te, 1],
                                     msq[:, ts:te])
                return mv, first

            def gn_affine(b, mv, ts=0, te=NT):
                """fused group-avg+broadcast matmul, then per-channel A/B
                for tiles [ts, te)."""
                nw = (te - ts) * 2
                bc_ps = psq.tile([P, 512], f32, tag="qkv")
                nc.tensor.matmul(bc_ps[:, :nw], gg_sb[:, :], mv[:, ts:te, :],
                                 start=True, stop=True)
                bc = small.tile([P, NT, 2], f32, tag="bcs")
                nc.vector.tensor_copy(bc[:, ts:te, :], bc_ps[:, 0:nw])
                vb = small.tile([P, NT], f32, tag="vb")
                nc.vector.tensor_mul(vb[:, ts:te], bc[:, ts:te, 0], bc[:, ts:te, 0])
                nc.vector.tensor_sub(vb[:, ts:te], bc[:, ts:te, 1], vb[:, ts:te])
                nc.vector.tensor_scalar_add(vb[:, ts:te], vb[:, ts:te], EPS)
                # rstd = rsqrt(var+eps): fast-inverse-sqrt + 1 Newton step
                # (all-DVE: keeps Sqrt off ACT so it never evicts Exp)
                ii = small.tile([P, NT], mybir.dt.int32, tag="ii")
                nc.vector.tensor_scalar(
                    out=ii[:, ts:te], in0=vb.bitcast(mybir.dt.int32)[:, ts:te],
                    scalar1=1, scalar2=None, op0=ALU.arith_shift_right)
                nc.vector.tensor_tensor(ii[:, ts:te], magic_sb[:, ts:te],
                                        ii[:, ts:te], op=ALU.subtract)
                y0 = ii.bitcast(f32)
                yt = small.tile([P, NT], f32, tag="yt")
                y1 = small.tile([P, NT], f32, tag="y1")
                nc.vector.tensor_mul(yt[:, ts:te], vb[:, ts:te], y0[:, ts:te])
                nc.vector.tensor_mul(yt[:, ts:te], yt[:, ts:te], y0[:, ts:te])
                nc.vector.tensor_scalar(out=yt[:, ts:te], in0=yt[:, ts:te],
                                        scalar1=-0.5, scalar2=1.5,
                                        op0=ALU.mult, op1=ALU.add)
                nc.vector.tensor_mul(y1[:, ts:te], y0[:, ts:te], yt[:, ts:te])
                tmp = small.tile([P, NT], f32, tag="tmpab")
                nc.vector.tensor_mul(a_all[:, b, ts:te], y1[:, ts:te],
                                     gab_sb[:, 0, ts:te])
                nc.vector.tensor_mul(tmp[:, ts:te], bc[:, ts:te, 0],
                                     a_all[:, b, ts:te])
                nc.vector.tensor_sub(b_all[:, b, ts:te], gab_sb[:, 1, ts:te],
                                     tmp[:, ts:te])

            def apply_h(b, after=None, h=None, ts=0, te=NT):
                """h = x*A + B (fp8), all-DVE. (NOT on ACT: the Activation
                engine has exec-queue depth 0, so an apply waiting on the
                DVE affine chain head-blocks the whole exp stream -- cost
                ~5us/iter, measured. DVE's depth-8 queue absorbs it.)"""
                if h is None:
                    h = hp.tile([P, NT, HW], f8, tag="h")
                last = None
                for t in range(ts, te):
                    last = nc.vector.tensor_scalar(
                        out=h[:, t, :], in0=x_sb[:, b, t, :],
                        scalar1=a_all[:, b, t:t + 1],
                        scalar2=b_all[:, b, t:t + 1],
                        op0=ALU.mult, op1=ALU.add,
                    )
                    if after is not None:
                        tile.add_dep_helper(last.ins, after.ins, sync=False,
                                            reason="apply after prev tmp")
                return h, last

            def gn_full_batched(b, after=None):
                """prologue GN in two 2-tile batches so apply(t0,t1) lands
                as soon as those x tiles arrive (qc's first DR pass only
                needs h tiles 0-1)."""
                mv = small.tile([P, NT, 2], f32, tag="mv")
                h = hp.tile([P, NT, HW], f8, tag="h")
                mv, first = gn_stats(b, after=after, mv=mv, ts=0, te=2)
                gn_affine(b, mv, 0, 2)
                _, last = apply_h(b, h=h, ts=0, te=2)
                gn_stats(b, mv=mv, ts=2, te=NT)
                gn_affine(b, mv, 2, NT)
                _, last = apply_h(b, h=h, ts=2, te=NT)
                return h, last, first

            def qc_proj(b, h_sb):
                """qc = A^T h (fp8 [d, hw], 8x-scaled); psum->sbuf on ACT."""
                qc_sb = qcp.tile([P, NT, HW], f8, tag="qc")
                for dm in range(NT):
                    ps0 = psq.tile([P, 512], f32, tag="qkv")
                    ps1 = psq.tile([P, 512], f32, tag="qkv")
                    for kp in range(NP):
                        lhsT = wqc_sb[:, 2 * kp:2 * kp + 2, dm * P:(dm + 1) * P]
                        st, sp = (kp == 0), (kp == NP - 1)
                        nc.tensor.matmul(ps0[:, :], lhsT,
                                         h_sb[:, 2 * kp:2 * kp + 2, 0:512],
                                         start=st, stop=sp, perf_mode=DR)
                        nc.tensor.matmul(ps1[:, :], lhsT,
                                         h_sb[:, 2 * kp:2 * kp + 2, 512:1024],
                                         start=st, stop=sp, perf_mode=DR)
                    nc.scalar.copy(qc_sb[:, dm, 0:512], ps0[:, :])
                    nc.scalar.copy(qc_sb[:, dm, 512:1024], ps1[:, :])
                return qc_sb

            def vt_proj(b, h_sb, split_copies=False):
                """vt' = (Wv Wt)^T h, transposed [hw, d] fp8 (8x-scaled);
                psum->sbuf on ACT (alternating with DVE for the deferred
                vt'(0), whose copies would otherwise stack up behind qc(1)'s
                in iteration 0's ACT queue)."""
                vt_sb = vtp.tile([P, NJ, C], f8, tag="vt")
                for jm in range(NJ):
                    ps = psq.tile([P, 512], f32, tag="qkv")
                    for kp in range(NP):
                        nc.tensor.matmul(
                            ps[:, :],
                            h_sb[:, 2 * kp:2 * kp + 2, jm * P:(jm + 1) * P],
                            wvt_sb[:, 2 * kp:2 * kp + 2, :],
                            start=(kp == 0), stop=(kp == NP - 1), perf_mode=DR,
                        )
                    if split_copies and jm % 2 == 0:
                        nc.vector.tensor_copy(vt_sb[:, jm, :], ps[:, :])
                    else:
                        nc.scalar.copy(vt_sb[:, jm, :], ps[:, :])
                return vt_sb

            # ---------------- prologue ----------------
            # vt'(0) is NOT issued here: it would delay scores(0) by 3.4us of
            # PE time; it slots into iteration 0 between qc(1) and vt'(1),
            # where its ACT copies run after exp(0) drains.
            # x[1] loads fire once GN(0) is underway (x[0] fully landed soon
            # after); x[2]/x[3] once GN(1) starts: keeps the wire exclusive
            # to whatever the pipeline needs next.
            h0_sb, last_apply, stats0_first = gn_full_batched(0)
            load_x(1, (nc.sync, nc.scalar), stats0_first)

            qc_next = qc_proj(0, h0_sb)

            h_next, last_apply, stats1_first = gn_full_batched(1)
            load_x(2, (nc.gpsimd, nc.sync), stats1_first)
            load_x(3, (nc.sync, nc.gpsimd), stats1_first)

            vt_next = None
            h_cur = h0_sb
            last_tmp = None
            apply_pending = None

            # ---------------- main loop ----------------
            mv_next = None
            for b in range(NB):
                qc_sb = qc_next
                vt_sb = vt_next

                # apply(b+1) at the DVE-queue HEAD of iter b (affine ran at
                # iter b-1's tail; the recycled h-buffer's readers finished
                # last iteration, so this runs immediately and h tiles 0-1
                # land by ~+5 -- before qc(b+1) reads them at +6.8. At the
                # TAIL it finished ~+9 into this iter and stalled qc ~5us.)
                if apply_pending is not None:
                    h_next, last_apply = apply_h(apply_pending, after=last_tmp)
                    apply_pending = None

                # iter-1: stats(3) next at the head (x[3] landed long ago;
                # fills DVE idle while exp(1) produces the den-tree inputs)
                if b == 1 and b + 2 < NB:
                    mv_next, _ = gn_stats(b + 2, after=last_apply)

                # ---- scores transposed (ST[j,i] = sum_d h[d,j] qc[d,i])
                #      + exp (ACT, fp8 out with -2 offset) ----
                est_sb = estp.tile([P, NJ, HW], f8e5, tag="est")
                for jm in range(NJ):
                    st_ps = psb.tile([P, HW], f32, tag="big")
                    for kp in range(NP):
                        lhsT = h_cur[:, 2 * kp:2 * kp + 2, jm * P:(jm + 1) * P]
                        st, sp = (kp == 0), (kp == NP - 1)
                        nc.tensor.matmul(st_ps[:, 0:512], lhsT,
                                         qc_sb[:, 2 * kp:2 * kp + 2, 0:512],
                                         start=st, stop=sp, perf_mode=DR)
                        nc.tensor.matmul(st_ps[:, 512:1024], lhsT,
                                         qc_sb[:, 2 * kp:2 * kp + 2, 512:1024],
                                         start=st, stop=sp, perf_mode=DR)
                    nc.scalar.activation(
                        out=est_sb[:, jm, :], in_=st_ps[:, :], func=ACT.Exp,
                        scale=CINV / WSC, bias=eoff_sb[:, 0:1],
                    )

                # ---- softmax denominator tree (bf16 out): level-1 split
                # 2 adds DVE / 2 adds GpSimd (idle early-iteration) ----
                s0 = sump.tile([P, HW], bf16, tag="s0")
                s1 = sump.tile([P, HW], bf16, tag="s1")
                s2 = sump.tile([P, HW], bf16, tag="s2")
                s3 = sump.tile([P, HW], bf16, tag="s3")
                nc.vector.tensor_add(s0[:, :], est_sb[:, 0, :], est_sb[:, 1, :])
                nc.gpsimd.tensor_add(s1[:, :], est_sb[:, 2, :], est_sb[:, 3, :])
                nc.vector.tensor_add(s2[:, :], est_sb[:, 4, :], est_sb[:, 5, :])
                nc.gpsimd.tensor_add(s3[:, :], est_sb[:, 6, :], est_sb[:, 7, :])
                nc.vector.tensor_add(s0[:, :], s0[:, :], s1[:, :])
                nc.vector.tensor_add(s2[:, :], s2[:, :], s3[:, :])
                nc.vector.tensor_add(s0[:, :], s0[:, :], s2[:, :])

                # ---- qc, vt' for sample b+1 fill TensorE while ACT exps ----
                if b + 1 < NB:
                    qc_next = qc_proj(b + 1, h_next)
                if b == 0:
                    vt_sb = vt_proj(0, h0_sb, split_copies=True)
                if b + 1 < NB:
                    vt_next = vt_proj(b + 1, h_next)

                # ---- PV fp8 DoubleRow over jm pairs: psum IS the final
                #      pre-residual activation (out-proj fused into vt').
                #      The den partition-reduce slots in AFTER PV cm0: rep is
                #      first needed by tmp(cm0), so this buys the DVE den
                #      tree ~2.3us of extra slack before the PE blocks on it.
                last_tmp = None
                rep = repp.tile([P, HW], f32, tag="rep")
                last_sample = b == NB - 1

                def den_recip():
                    rs0 = rsp.tile([P, 512], f32, tag="rs")
                    rs1 = rsp.tile([P, 512], f32, tag="rs")
                    nc.tensor.matmul(rs0[:, :], ones_sb[:, :], s0[:, 0:512],
                                     start=True, stop=True)
                    nc.tensor.matmul(rs1[:, :], ones_sb[:, :],
                                     s0[:, 512:1024], start=True, stop=True)
                    nc.vector.reciprocal_approx_fast(out=rep[:, 0:512],
                                                     in_=rs0[:, :])
                    nc.vector.reciprocal_approx_fast(out=rep[:, 512:1024],
                                                     in_=rs1[:, :])

                if last_sample:
                    # no next-sample to protect: den-reduce up front, and
                    # process each tile in token-halves so the tail out-DMAs
                    # (12us wire latency each) start as early as possible
                    den_recip()
                for cm in range(NT):
                    o_ps = psb.tile([P, HW], f32, tag="big")
                    for hh0, hh1 in ((0, 512), (512, 1024)) if last_sample \
                            else ((0, 1024),):
                        for t in range(NJP):
                            lhsT = vt_sb[:, 2 * t:2 * t + 2,
                                         cm * P:(cm + 1) * P]
                            st, sp = (t == 0), (t == NJP - 1)
                            if last_sample:
                                nc.tensor.matmul(
                                    o_ps[:, hh0:hh1], lhsT,
                                    est_sb[:, 2 * t:2 * t + 2, hh0:hh1],
                                    start=st, stop=sp, perf_mode=DR)
                            else:
                                nc.tensor.matmul(
                                    o_ps[:, 0:512], lhsT,
                                    est_sb[:, 2 * t:2 * t + 2, 0:512],
                                    start=st, stop=sp, perf_mode=DR)
                                nc.tensor.matmul(
                                    o_ps[:, 512:1024], lhsT,
                                    est_sb[:, 2 * t:2 * t + 2, 512:1024],
                                    start=st, stop=sp, perf_mode=DR)
                        if not last_sample and cm == 0:
                            # den partition-reduce after PV cm0 (see note)
                            den_recip()
                        # tmp = psum * rep on DVE; residual add on GpSimd
                        # (DVE for the last sample: shorter tail); DMA issues
                        # on the idle Sync engine (~600ns each)
                        if last_sample:
                            tmv = tmpp.tile([P, HW], f32, tag="tmp")
                            nc.vector.tensor_mul(tmv[:, hh0:hh1],
                                                 o_ps[:, hh0:hh1],
                                                 rep[:, hh0:hh1])
                            out_t = outp.tile([P, HW], f32, tag="out")
                            rr = [(hh0, hh1)]
                        else:
                            tmv = tmpp.tile([P, HW], f32, tag="tmp")
                            last_tmp = nc.vector.tensor_mul(
                                tmv[:, :], o_ps[:, :], rep[:, :])
                            out_t = outp.tile([P, HW], f32, tag="out")
                            rr = [(0, 512), (512, 1024)]
                        for h0, h1 in rr:
                            if zero_out_bias:
                                eng = nc.vector if last_sample else nc.gpsimd
                                eng.tensor_add(out_t[:, h0:h1],
                                               tmv[:, h0:h1],
                                               x_sb[:, b, cm, h0:h1])
                            else:
                                nc.vector.scalar_tensor_tensor(
                                    out=out_t[:, h0:h1],
                                    in0=tmv[:, h0:h1],
                                    scalar=btp_sb[:, cm:cm + 1],
                                    in1=x_sb[:, b, cm, h0:h1],
                                    op0=ALU.add, op1=ALU.add,
                                )
                            if last_sample:
                                deng = (nc.sync, nc.scalar, nc.gpsimd)[
                                    (2 * cm + h0 // 512) % 3]
                            else:
                                deng = nc.sync
                            deng.dma_start(
                                out=out_d[b, cm * P:(cm + 1) * P, h0:h1],
                                in_=out_t[:, h0:h1])

                # tail: stats(2) on iter-0 (x[2] lands mid-iter-0), then
                # affine(b+2); its bc matmul runs after PV(b) on the PE.
                # The APPLY is deferred to the next iteration's DVE head.
                if b + 2 < NB:
                    h_cur = h_next
                    if b == 0:
                        mv_next, _ = gn_stats(b + 2, after=last_apply)
                    gn_affine(b + 2, mv_next)
                    apply_pending = b + 2
                else:
                    h_cur = h_next

    nc.compile()
    return nc


def prep_inputs(inputs):
    """Host-side prep: per-core in_maps with fused, pre-laid-out weights."""
    e4 = ml_dtypes.float8_e4m3
    x = np.ascontiguousarray(np.asarray(inputs["x"], dtype=np.float32)).reshape(
        B_FULL, C, HW
    )

    Wq = np.asarray(inputs["Wq"], dtype=np.float64)
    Wk = np.asarray(inputs["Wk"], dtype=np.float64)
    Wv = np.asarray(inputs["Wv"], dtype=np.float64)
    Wt = np.asarray(inputs["Wt"], dtype=np.float64)
    A = (WSC * (Wq @ Wk.T)).astype(np.float32)   # scores = h^T A h / 8
    Bm = (WSC * (Wv @ Wt)).astype(np.float32)    # out = (B^T h)^T P / 8

    def wprep(w):
        # [C, C] -> [P, NT, C]  (lhsT slices w[kc*128+p, d])
        return np.ascontiguousarray(
            np.asarray(w, dtype=np.float32).reshape(NT, P, C).transpose(1, 0, 2)
        ).astype(e4)

    def cols(v):
        # [C] -> [P, NT]
        return np.ascontiguousarray(
            np.asarray(v, dtype=np.float32).reshape(NT, P).T
        )

    gab = np.stack([cols(inputs["gn_scale"]), cols(inputs["gn_bias"])], axis=1)
    gg = np.zeros((P, P), np.float32)
    for p in range(P):
        gg[p, (p // GS) * GS:(p // GS + 1) * GS] = 1.0 / GS

    btp = (np.asarray(inputs["bt"], dtype=np.float64)
           + Wt.T @ np.asarray(inputs["bv"], dtype=np.float64)).astype(np.float32)

    shared = {
        "wqc": wprep(A), "wvt": wprep(Bm),
        "gn_ab": np.ascontiguousarray(gab), "gg": gg,
    }
    if np.any(btp != 0):
        shared["btp"] = cols(btp)
    in_maps = []
    for c_id in range(N_CORES):
        m = dict(shared)
        m["x"] = np.ascontiguousarray(x[c_id * NB:(c_id + 1) * NB])
        in_maps.append(m)
    return in_maps


_NC_CACHE = {}


def get_nc(zero_out_bias=True):
    key = (zero_out_bias,)
    if key not in _NC_CACHE:
        _NC_CACHE[key] = build_nc(zero_out_bias=zero_out_bias)
    return _NC_CACHE[key]


def _kernel_numpy(inputs):
    """Pure-numpy fallback, only for nonzero bq/bk (never hit by this
    problem's inputs -- setup_inputs() uses zero biases)."""
    x = np.asarray(inputs["x"], dtype=np.float64)
    B, C_, H_, W_ = x.shape
    g = x.reshape(B, NGROUPS, C_ // NGROUPS, H_, W_)
    mean = g.mean(axis=(2, 3, 4), keepdims=True)
    var = g.var(axis=(2, 3, 4), keepdims=True)
    hn = ((g - mean) / np.sqrt(var + EPS)).reshape(B, C_, H_, W_)
    hn = hn * np.asarray(inputs["gn_scale"], np.float64)[None, :, None, None] \
        + np.asarray(inputs["gn_bias"], np.float64)[None, :, None, None]

    def nin(h, Wm, bm):
        return np.einsum("bchw,cd->bdhw", h, np.asarray(Wm, np.float64)) \
            + np.asarray(bm, np.float64)[None, :, None, None]

    q = nin(hn, inputs["Wq"], inputs["bq"])
    k = nin(hn, inputs["Wk"], inputs["bk"])
    v = nin(hn, inputs["Wv"], inputs["bv"])
    w = np.einsum("bchw,bcij->bhwij", q, k) * (C_ ** -0.5)
    w = w.reshape(B, H_, W_, H_ * W_)
    w = np.exp(w - w.max(axis=-1, keepdims=True))
    w = (w / w.sum(axis=-1, keepdims=True)).reshape(B, H_, W_, H_, W_)
    hh = np.einsum("bhwij,bcij->bchw", w, v)
    hh = nin(hh, inputs["Wt"], inputs["bt"])
    return (hh + x).astype(np.float32)


def run(inputs, trace=False):
    from concourse.bass_utils import run_bass_kernel_spmd

    if not (np.all(np.asarray(inputs["bq"]) == 0)
            and np.all(np.asarray(inputs["bk"]) == 0)):
        return _kernel_numpy(inputs), None

    in_maps = prep_inputs(inputs)
    nc = get_nc(zero_out_bias="btp" not in in_maps[0])
    res = run_bass_kernel_spmd(
        nc, in_maps, core_ids=list(range(N_CORES)), trace=trace
    )
    out = np.concatenate([np.asarray(r["out"]) for r in res.results], axis=0)
    return out.reshape(B_FULL, C, H, W), res


def kernel(**inputs):
    out, _ = run(inputs, trace=False)
    return out


# revision 44
# speedup vs baseline: 1.1634x; 1.0642x over previous
"""AttnBlock (GroupNorm -> QKV 1x1 -> full attention over 1024 tokens -> out-proj
+ residual) for x [32, 512, 32, 32] f32, distributed data-parallel over 8
NeuronCores (4 samples per core, weights replicated).

Per-core single-NC Bass/Tile kernel, v2: weight-fusion + all-fp8 DoubleRow.

Algebraic restructuring (host-side, exact in f32):
  - scores  = (Wq h)^T (Wk h) = h^T A h with A = Wq @ Wk.T  -> ONE fused
    "qc" projection (qc = A^T h) replaces the separate Q and K projections.
  - out-proj fused into V: vt' = (Wv @ Wt)^T h gives
    out = vt'^T P_norm  directly, so the PV matmul's PSUM already holds the
    final pre-residual activation -- the separate out-projection disappears.
  - A and B=Wv@Wt are pre-scaled by 8 on the host so their entries clear the
    fp8e4 subnormal range; the 8x cancels via exp-scale (CINV/8) on the
    score side and via rep = 1/(8*den) on the PV side.

Per-sample PE work drops 82944 -> ~50200 col-cycles (scores 16384 + PV-DR
16384 + qc 8192 + vt' 8192 + den 1024): every matmul issues moving columns at
1/cycle regardless of dtype, so fp8 DoubleRow's 2x comes purely from halving
the pass count (contraction 256 rows/pass).

  - est = exp(s*c - 4.5) stored fp8e5 (e5m2: scores empirically reach 13.8
    sigma, far past fp8e4's e^11.7 dynamic range; e5m2 spans it easily and
    softmax normalization cancels most of its 2-mantissa-bit noise -- host
    emulation: 0.0078 rel err vs 0.0068 for bf16 est). The offset cancels
    exactly in softmax. PV runs fp8 DoubleRow (e4m3 vt x e5m2 est) over
    jm-pair passes.
  - softmax denominator: DVE pairwise-add tree over the 8 fp8 exp tiles
    (bf16 out, 2x DVE rate), one 8.0-matrix bf16 matmul reduces partitions
    and replicates 8*den; rep = 1/(8 den) via reciprocal_approx_fast.
  - residual: tmp = PV_psum * rep on DVE, out = tmp + x on GpSimd (idle
    otherwise; DVE for the last sample to shorten the tail), DMA per half.
  - engine balance per sample: PE ~20.9us, DVE ~19.8us (GN stats/apply, den
    tree, recip, tmp), ACT ~17.4us (exp + qc/vt' psum->sbuf fp8 copies),
    GpSimd ~9us (residual adds + out DMA issue).
  - prologue: x[0] owns all three DMA rings before weights/x[1..3] queue up;
    GN(0) stats start per-tile as x[0] tiles land.
"""

import os
import sys

import numpy as np

sys.path.insert(0, "/opt/trn_rl_repo")

import ml_dtypes  # noqa: E402

import concourse.bass as bass  # noqa: E402
import concourse.tile as tile  # noqa: E402
from concourse import bacc, mybir  # noqa: E402

P = 128
B_FULL, C, H, W = 32, 512, 32, 32
HW = H * W            # 1024 tokens
N_CORES = 8
NB = B_FULL // N_CORES  # 4 samples per core
NT = C // P           # 4 channel tiles
NP = NT // 2          # 2 DoubleRow channel-tile pairs
NJ = HW // P          # 8 token tiles
NJP = NJ // 2         # 4 DoubleRow token-tile pairs
NGROUPS = 32
GS = C // NGROUPS     # 16 channels per group
EPS = 1e-6
CINV = float(C) ** -0.5
WSC = 8.0             # fused weights pre-scaled by 8 (fp8 subnormal guard)
EOFF = -4.5           # exp offset: est = exp(s*c - 4.5), cancels in softmax

f32 = mybir.dt.float32
bf16 = mybir.dt.bfloat16
f8 = mybir.dt.float8e4
f8e5 = mybir.dt.float8e5
ALU = mybir.AluOpType
ACT = mybir.ActivationFunctionType
DR = mybir.MatmulPerfMode.DoubleRow


def build_nc(zero_out_bias=True):
    """Build the single-core Bass graph (SPMD: same graph on all 8 cores).

    zero_out_bias: fused output bias bt' = bt + Wt^T bv is all-zero (true for
    this problem), so the residual add drops the bias column.
    """
    nc = bacc.Bacc("TRN2", target_bir_lowering=False, debug=False)

    x_d = nc.dram_tensor("x", [NB, C, HW], f32, kind="ExternalInput")
    wqc_d = nc.dram_tensor("wqc", [P, NT, C], f8, kind="ExternalInput")
    wvt_d = nc.dram_tensor("wvt", [P, NT, C], f8, kind="ExternalInput")
    # gn affine columns: [:, 0, :]=gamma, [:, 1, :]=beta
    gab_d = nc.dram_tensor("gn_ab", [P, 2, NT], f32, kind="ExternalInput")
    # block-diagonal group-average matrix: GG[k,p] = 1/16 iff k//16 == p//16
    gg_d = nc.dram_tensor("gg", [P, P], f32, kind="ExternalInput")
    if not zero_out_bias:
        btp_d = nc.dram_tensor("btp", [P, NT], f32, kind="ExternalInput")
    out_d = nc.dram_tensor("out", [NB, C, HW], f32, kind="ExternalOutput")

    with tile.TileContext(nc) as tc:
        with (
            tc.tile_pool(name="consts", bufs=1) as consts,
            tc.tile_pool(name="hp", bufs=2) as hp,
            tc.tile_pool(name="qcp", bufs=2) as qcp,
            tc.tile_pool(name="vtp", bufs=2) as vtp,
            tc.tile_pool(name="est", bufs=1) as estp,
            tc.tile_pool(name="sump", bufs=1) as sump,
            tc.tile_pool(name="repp", bufs=1) as repp,
            # tmpp=4: with only 2 bufs, tmp(cm3)'s alloc waits the GpSimd
            # residual-add of cm1, putting the slow GpSimd chain on the
            # critical path into scores(b+1) via the psb rotation (measured
            # ~5us/iter stall).
            tc.tile_pool(name="tmpp", bufs=4) as tmpp,
            tc.tile_pool(name="outp", bufs=4) as outp,
            tc.tile_pool(name="small", bufs=2) as small,
            # psb 2x[P,1024] (4) + psq 2x[P,512] (2) + rsp 2x[P,512] (2)
            # = all 8 banks. The den-reduce psums get their OWN pool (rsp):
            # inside the psq rotation, qc(b+1)'s first psum alloc waits on
            # recip(b), serializing the whole next-sample projection phase
            # behind the exp->gpsimd-tree->den-red->recip latency chain
            # (~10us/iter, measured).
            tc.tile_pool(name="psb", bufs=2, space="PSUM") as psb,
            tc.tile_pool(name="psq", bufs=2, space="PSUM") as psq,
            tc.tile_pool(name="rsp", bufs=2, space="PSUM") as rsp,
        ):
            x_sb = consts.tile([P, NB, NT, HW], f32, tag="x")
            wqc_sb = consts.tile([P, NT, C], f8, tag="wqc")
            wvt_sb = consts.tile([P, NT, C], f8, tag="wvt")
            gab_sb = consts.tile([P, 2, NT], f32, tag="gab")
            gg_sb = consts.tile([P, P], f32, tag="gg")
            if not zero_out_bias:
                btp_sb = consts.tile([P, NT], f32, tag="btp")

            # gg+gab first on gpsimd (tiny, needed by affine(0)); then x[0]
            # halves (2KB contiguous per partition row) own ALL THREE rings.
            # DMA engines pull from all queued transfers CONCURRENTLY (not
            # ring-order), so x[1..3] loads are semaphore-GATED behind GN
            # progress below -- otherwise they steal ~2/3 of the wire and
            # x[0] (which gates the whole GN(0)->qc(0) prologue chain) lands
            # ~7us late (measured).
            # Each DMA transfer is serviced by ~one engine at ~21 B/ns, so
            # wire parallelism = in-flight transfer count. Transfers must
            # keep 2KB-contiguous rows and FULL 128 partitions (64-partition
            # transfers run at half rate -- measured), so a 2MB sample maxes
            # out at 8 transfers = ~12us. x[1..3] are semaphore-gated behind
            # GN progress so they don't steal the wire from x[0].
            nc.gpsimd.dma_start(out=gg_sb[:, :], in_=gg_d[:, :])
            nc.gpsimd.dma_start(out=gab_sb[:, :, :], in_=gab_d[:, :, :])
            engs = (nc.sync, nc.scalar, nc.gpsimd)
            qi = 0
            for t in range(NT):
                for h0 in (0, 512):
                    engs[qi % 3].dma_start(
                        out=x_sb[:, 0, t, h0:h0 + 512],
                        in_=x_d[0, t * P:(t + 1) * P, h0:h0 + 512])
                    qi += 1
            # weights in 2 transfers each, right behind x[0]
            for t2 in (0, 2):
                nc.sync.dma_start(out=wqc_sb[:, t2:t2 + 2, :],
                                  in_=wqc_d[:, t2:t2 + 2, :])
                nc.scalar.dma_start(out=wvt_sb[:, t2:t2 + 2, :],
                                    in_=wvt_d[:, t2:t2 + 2, :])
            if not zero_out_bias:
                nc.gpsimd.dma_start(out=btp_sb[:, :], in_=btp_d[:, :])

            def load_x(b, eng_list, after):
                """Issue sample b's x as 8 half-tile transfers (full rate)
                round-robined over eng_list, gated behind `after`."""
                qi = 0
                for t in range(NT):
                    for h0 in (0, 512):
                        eng = eng_list[qi % len(eng_list)]
                        qi += 1
                        dd = eng.dma_start(
                            out=x_sb[:, b, t, h0:h0 + 512],
                            in_=x_d[b, t * P:(t + 1) * P, h0:h0 + 512])
                        if after is not None:
                            tile.add_dep_helper(dd.ins, after.ins, sync=True,
                                                reason="x load gated on GN progress")

            # 8.0-matrix: reduces den partials across partitions AND folds the
            # 8x fused-weight scale (rep = 1/(8*den))
            ones_sb = consts.tile([P, P], bf16, tag="ones")
            nc.vector.memset(ones_sb[:, :], WSC)
            magic_sb = consts.tile([P, NT], mybir.dt.int32, tag="magic")
            nc.vector.memset(magic_sb[:, :], 0x5F3759DF)
            # per-partition exp-offset column (bias operand for the Exp calls)
            eoff_sb = consts.tile([P, 1], f32, tag="eoff")
            nc.vector.memset(eoff_sb[:, :], EOFF)
            # dummy Exp: pulls the ACT Exp-table load into the idle prologue
            expwarm = small.tile([P, 1], f32, tag="expwarm")
            nc.scalar.activation(out=expwarm[:, :], in_=ones_sb[:, 0:1],
                                 func=ACT.Exp, scale=CINV / WSC,
                                 bias=eoff_sb[:, 0:1])

            a_all = consts.tile([P, NB, NT], f32, tag="a_all")
            b_all = consts.tile([P, NB, NT], f32, tag="b_all")

            # PE warm-up on gg (earliest-arriving tensor): releases the HAM
            # clock-gate before the real stream.
            warm_ps = psq.tile([P, 512], f32, tag="qkv")
            for w in range(5):
                nc.tensor.matmul(
                    warm_ps[:, 0:128], gg_sb[:, 0:128], gg_sb[:, 0:128],
                    start=(w == 0), stop=(w == 4),
                )

            def gn_stats(b, after=None, mv=None, ts=0, te=NT):
                """bn stats -> per-channel (mean, Ex2) in mv[:, ts:te, :].
                GN groups (16ch) never span 128-channel tiles, so the whole
                GN chain is per-tile independent -- the prologue exploits
                this with 2-tile batches that chase the x DMAs."""
                if mv is None:
                    mv = small.tile([P, NT, 2], f32, tag="mv")
                nsub = 2
                step = HW // nsub
                first = None
                for t in range(ts, te):
                    st6 = small.tile([P, nsub, 6], f32, tag="st6")
                    for q in range(nsub):
                        iq = nc.vector.bn_stats(
                            out=st6[:, q, :],
                            in_=x_sb[:, b, t, q * step:(q + 1) * step])
                        if first is None:
                            first = iq
                        if after is not None:
                            tile.add_dep_helper(iq.ins, after.ins, sync=False,
                                                reason="gn stats after prev apply")
                    nc.vector.bn_aggr(out=mv[:, t, :], in_=st6[:, :, :])
                msq = small.tile([P, NT], f32, tag="msq")
                nc.vector.tensor_mul(msq[:, ts:te], mv[:, ts:te, 0],
                                     mv[:, ts:te, 0])
                nc.vector.tensor_add(mv[:, ts:te, 1], mv[:, ts:te, 1],
                                     msq[:, ts:te])
                return mv, first

            def gn_affine(b, mv, ts=0, te=NT):
                """fused group-avg+broadcast matmul, then per-channel A/B
                for tiles [ts, te)."""
                nw = (te - ts) * 2
                bc_ps = psq.tile([P, 512], f32, tag="qkv")
                nc.tensor.matmul(bc_ps[:, :nw], gg_sb[:, :], mv[:, ts:te, :],
                                 start=True, stop=True)
                bc = small.tile([P, NT, 2], f32, tag="bcs")
                nc.vector.tensor_copy(bc[:, ts:te, :], bc_ps[:, 0:nw])
                vb = small.tile([P, NT], f32, tag="vb")
                nc.vector.tensor_mul(vb[:, ts:te], bc[:, ts:te, 0], bc[:, ts:te, 0])
                nc.vector.tensor_sub(vb[:, ts:te], bc[:, ts:te, 1], vb[:, ts:te])
                nc.vector.tensor_scalar_add(vb[:, ts:te], vb[:, ts:te], EPS)
                # rstd = rsqrt(var+eps): fast-inverse-sqrt + 1 Newton step
                # (all-DVE: keeps Sqrt off ACT so it never evicts Exp)
                ii = small.tile([P, NT], mybir.dt.int32, tag="ii")
                nc.vector.tensor_scalar(
                    out=ii[:, ts:te], in0=vb.bitcast(mybir.dt.int32)[:, ts:te],
                    scalar1=1, scalar2=None, op0=ALU.arith_shift_right)
                nc.vector.tensor_tensor(ii[:, ts:te], magic_sb[:, ts:te],
                                        ii[:, ts:te], op=ALU.subtract)
                y0 = ii.bitcast(f32)
                yt = small.tile([P, NT], f32, tag="yt")
                y1 = small.tile([P, NT], f32, tag="y1")
                nc.vector.tensor_mul(yt[:, ts:te], vb[:, ts:te], y0[:, ts:te])
                nc.vector.tensor_mul(yt[:, ts:te], yt[:, ts:te], y0[:, ts:te])
                nc.vector.tensor_scalar(out=yt[:, ts:te], in0=yt[:, ts:te],
                                        scalar1=-0.5, scalar2=1.5,
                                        op0=ALU.mult, op1=ALU.add)
                nc.vector.tensor_mul(y1[:, ts:te], y0[:, ts:te], yt[:, ts:te])
                tmp = small.tile([P, NT], f32, tag="tmpab")
                nc.vector.tensor_mul(a_all[:, b, ts:te], y1[:, ts:te],
                                     gab_sb[:, 0, ts:te])
                nc.vector.tensor_mul(tmp[:, ts:te], bc[:, ts:te, 0],
                                     a_all[:, b, ts:te])
                nc.vector.tensor_sub(b_all[:, b, ts:te], gab_sb[:, 1, ts:te],
                                     tmp[:, ts:te])

            def apply_h(b, after=None, h=None, ts=0, te=NT):
                """h = x*A + B (fp8), all-DVE. (NOT on ACT: the Activation
                engine has exec-queue depth 0, so an apply waiting on the
                DVE affine chain head-blocks the whole exp stream -- cost
                ~5us/iter, measured. DVE's depth-8 queue absorbs it.)"""
                if h is None:
                    h = hp.tile([P, NT, HW], f8, tag="h")
                last = None
                for t in range(ts, te):
                    last = nc.vector.tensor_scalar(
                        out=h[:, t, :], in0=x_sb[:, b, t, :],
                        scalar1=a_all[:, b, t:t + 1],
                        scalar2=b_all[:, b, t:t + 1],
                        op0=ALU.mult, op1=ALU.add,
                    )
                    if after is not None:
                        tile.add_dep_helper(last.ins, after.ins, sync=False,
                                            reason="apply after prev tmp")
                return h, last

            def gn_full_batched(b, after=None):
                """prologue GN in two 2-tile batches so apply(t0,t1) lands
                as soon as those x tiles arrive (qc's first DR pass only
                needs h tiles 0-1)."""
                mv = small.tile([P, NT, 2], f32, tag="mv")
                h = hp.tile([P, NT, HW], f8, tag="h")
                mv, first = gn_stats(b, after=after, mv=mv, ts=0, te=2)
                gn_affine(b, mv, 0, 2)
                _, last = apply_h(b, h=h, ts=0, te=2)
                gn_stats(b, mv=mv, ts=2, te=NT)
                gn_affine(b, mv, 2, NT)
                _, last = apply_h(b, h=h, ts=2, te=NT)
                return h, last, first

            def qc_proj(b, h_sb):
                """qc = A^T h (fp8 [d, hw], 8x-scaled); psum->sbuf on ACT."""
                qc_sb = qcp.tile([P, NT, HW], f8, tag="qc")
                for dm in range(NT):
                    ps0 = psq.tile([P, 512], f32, tag="qkv")
                    ps1 = psq.tile([P, 512], f32, tag="qkv")
                    for kp in range(NP):
                        lhsT = wqc_sb[:, 2 * kp:2 * kp + 2, dm * P:(dm + 1) * P]
                        st, sp = (kp == 0), (kp == NP - 1)
                        nc.tensor.matmul(ps0[:, :], lhsT,
                                         h_sb[:, 2 * kp:2 * kp + 2, 0:512],
                                         start=st, stop=sp, perf_mode=DR)
                        nc.tensor.matmul(ps1[:, :], lhsT,
                                         h_sb[:, 2 * kp:2 * kp + 2, 512:1024],
                                         start=st, stop=sp, perf_mode=DR)
                    nc.scalar.copy(qc_sb[:, dm, 0:512], ps0[:, :])
                    nc.scalar.copy(qc_sb[:, dm, 512:1024], ps1[:, :])
                return qc_sb

            def vt_proj(b, h_sb, split_copies=False):
                """vt' = (Wv Wt)^T h, transposed [hw, d] fp8 (8x-scaled);
                psum->sbuf on ACT (alternating with DVE for the deferred
                vt'(0), whose copies would otherwise stack up behind qc(1)'s
                in iteration 0's ACT queue)."""
                vt_sb = vtp.tile([P, NJ, C], f8, tag="vt")
                for jm in range(NJ):
                    ps = psq.tile([P, 512], f32, tag="qkv")
                    for kp in range(NP):
                        nc.tensor.matmul(
                            ps[:, :],
                            h_sb[:, 2 * kp:2 * kp + 2, jm * P:(jm + 1) * P],
                            wvt_sb[:, 2 * kp:2 * kp + 2, :],
                            start=(kp == 0), stop=(kp == NP - 1), perf_mode=DR,
                        )
                    if split_copies and jm % 2 == 0:
                        nc.vector.tensor_copy(vt_sb[:, jm, :], ps[:, :])
                    else:
                        nc.scalar.copy(vt_sb[:, jm, :], ps[:, :])
                return vt_sb

            # ---------------- prologue ----------------
            # vt'(0) is NOT issued here: it would delay scores(0) by 3.4us of
            # PE time; it slots into iteration 0 between qc(1) and vt'(1),
            # where its ACT copies run after exp(0) drains.
            # x[1] loads fire once GN(0) is underway (x[0] fully landed soon
            # after); x[2]/x[3] once GN(1) starts: keeps the wire exclusive
            # to whatever the pipeline needs next. (GN is NOT tile-batched:
            # the affine chain is small-op-overhead-dominated, and doubling
            # it costs more than the earlier apply saves -- measured.)
            mv0, stats0_first = gn_stats(0)
            load_x(1, (nc.sync, nc.scalar), stats0_first)
            gn_affine(0, mv0)
            h0_sb, last_apply = apply_h(0)

            qc_next = qc_proj(0, h0_sb)

            mv1, stats1_first = gn_stats(1, after=last_apply)
            load_x(2, (nc.gpsimd, nc.sync), stats1_first)
            load_x(3, (nc.sync, nc.gpsimd), stats1_first)
            gn_affine(1, mv1)
            h_next, last_apply = apply_h(1)

            vt_next = None
            h_cur = h0_sb
            last_tmp = None

            # ---------------- main loop ----------------
            mv_next = None
            for b in range(NB):
                qc_sb = qc_next
                vt_sb = vt_next

                # iter>=1: GN stats for b+2 at the DVE-queue HEAD -- its x
                # landed long ago, it has no cross-engine deps, and it fills
                # the DVE idle while exp(b) produces the den-tree inputs.
                if b >= 1 and b + 2 < NB:
                    mv_next, _ = gn_stats(b + 2, after=last_apply)

                # ---- scores transposed (ST[j,i] = sum_d h[d,j] qc[d,i])
                #      + exp (ACT, fp8 out with -2 offset) ----
                est_sb = estp.tile([P, NJ, HW], f8e5, tag="est")
                for jm in range(NJ):
                    st_ps = psb.tile([P, HW], f32, tag="big")
                    for kp in range(NP):
                        lhsT = h_cur[:, 2 * kp:2 * kp + 2, jm * P:(jm + 1) * P]
                        st, sp = (kp == 0), (kp == NP - 1)
                        nc.tensor.matmul(st_ps[:, 0:512], lhsT,
                                         qc_sb[:, 2 * kp:2 * kp + 2, 0:512],
                                         start=st, stop=sp, perf_mode=DR)
                        nc.tensor.matmul(st_ps[:, 512:1024], lhsT,
                                         qc_sb[:, 2 * kp:2 * kp + 2, 512:1024],
                                         start=st, stop=sp, perf_mode=DR)
                    nc.scalar.activation(
                        out=est_sb[:, jm, :], in_=st_ps[:, :], func=ACT.Exp,
                        scale=CINV / WSC, bias=eoff_sb[:, 0:1],
                    )

                # ---- softmax denominator tree (bf16 out): level-1 split
                # 2 adds DVE / 2 adds GpSimd (idle early-iteration) ----
                s0 = sump.tile([P, HW], bf16, tag="s0")
                s1 = sump.tile([P, HW], bf16, tag="s1")
                s2 = sump.tile([P, HW], bf16, tag="s2")
                s3 = sump.tile([P, HW], bf16, tag="s3")
                nc.vector.tensor_add(s0[:, :], est_sb[:, 0, :], est_sb[:, 1, :])
                nc.gpsimd.tensor_add(s1[:, :], est_sb[:, 2, :], est_sb[:, 3, :])
                nc.vector.tensor_add(s2[:, :], est_sb[:, 4, :], est_sb[:, 5, :])
                nc.gpsimd.tensor_add(s3[:, :], est_sb[:, 6, :], est_sb[:, 7, :])
                nc.vector.tensor_add(s0[:, :], s0[:, :], s1[:, :])
                nc.vector.tensor_add(s2[:, :], s2[:, :], s3[:, :])
                nc.vector.tensor_add(s0[:, :], s0[:, :], s2[:, :])

                # ---- qc, vt' for sample b+1 fill TensorE while ACT exps ----
                if b + 1 < NB:
                    qc_next = qc_proj(b + 1, h_next)
                if b == 0:
                    vt_sb = vt_proj(0, h0_sb)
                if b + 1 < NB:
                    vt_next = vt_proj(b + 1, h_next)

                # ---- PV fp8 DoubleRow over jm pairs: psum IS the final
                #      pre-residual activation (out-proj fused into vt').
                #      The den partition-reduce slots in AFTER PV cm0: rep is
                #      first needed by tmp(cm0), so this buys the DVE den
                #      tree ~2.3us of extra slack before the PE blocks on it.
                last_tmp = None
                rep = repp.tile([P, HW], f32, tag="rep")
                last_sample = b == NB - 1

                def den_recip():
                    rs0 = rsp.tile([P, 512], f32, tag="rs")
                    rs1 = rsp.tile([P, 512], f32, tag="rs")
                    nc.tensor.matmul(rs0[:, :], ones_sb[:, :], s0[:, 0:512],
                                     start=True, stop=True)
                    nc.tensor.matmul(rs1[:, :], ones_sb[:, :],
                                     s0[:, 512:1024], start=True, stop=True)
                    nc.vector.reciprocal_approx_fast(out=rep[:, 0:512],
                                                     in_=rs0[:, :])
                    nc.vector.reciprocal_approx_fast(out=rep[:, 512:1024],
                                                     in_=rs1[:, :])

                if last_sample:
                    # no next-sample to protect: den-reduce up front, and
                    # process each tile in token-halves so the tail out-DMAs
                    # (12us wire latency each) start as early as possible
                    den_recip()
                for cm in range(NT):
                    o_ps = psb.tile([P, HW], f32, tag="big")
                    for hh0, hh1 in ((0, 512), (512, 1024)) if last_sample \
                            else ((0, 1024),):
                        for t in range(NJP):
                            lhsT = vt_sb[:, 2 * t:2 * t + 2,
                                         cm * P:(cm + 1) * P]
                            st, sp = (t == 0), (t == NJP - 1)
                            if last_sample:
                                nc.tensor.matmul(
                                    o_ps[:, hh0:hh1], lhsT,
                                    est_sb[:, 2 * t:2 * t + 2, hh0:hh1],
                                    start=st, stop=sp, perf_mode=DR)
                            else:
                                nc.tensor.matmul(
                                    o_ps[:, 0:512], lhsT,
                                    est_sb[:, 2 * t:2 * t + 2, 0:512],
                                    start=st, stop=sp, perf_mode=DR)
                                nc.tensor.matmul(
                                    o_ps[:, 512:1024], lhsT,
                                    est_sb[:, 2 * t:2 * t + 2, 512:1024],
                                    start=st, stop=sp, perf_mode=DR)
                        if not last_sample and cm == 0:
                            # den partition-reduce after PV cm0 (see note)
                            den_recip()
                        # tmp = psum * rep on DVE; residual add on GpSimd
                        # (DVE for the last sample: shorter tail); DMA issues
                        # on the idle Sync engine (~600ns each)
                        if last_sample:
                            tmv = tmpp.tile([P, HW], f32, tag="tmp")
                            nc.vector.tensor_mul(tmv[:, hh0:hh1],
                                                 o_ps[:, hh0:hh1],
                                                 rep[:, hh0:hh1])
                            out_t = outp.tile([P, HW], f32, tag="out")
                            rr = [(hh0, hh1)]
                        else:
                            tmv = tmpp.tile([P, HW], f32, tag="tmp")
                            last_tmp = nc.vector.tensor_mul(
                                tmv[:, :], o_ps[:, :], rep[:, :])
                            out_t = outp.tile([P, HW], f32, tag="out")
                            rr = [(0, 512), (512, 1024)]
                        for h0, h1 in rr:
                            if zero_out_bias:
                                eng = nc.vector if last_sample else nc.gpsimd
                                eng.tensor_add(out_t[:, h0:h1],
                                               tmv[:, h0:h1],
                                               x_sb[:, b, cm, h0:h1])
                            else:
                                nc.vector.scalar_tensor_tensor(
                                    out=out_t[:, h0:h1],
                                    in0=tmv[:, h0:h1],
                                    scalar=btp_sb[:, cm:cm + 1],
                                    in1=x_sb[:, b, cm, h0:h1],
                                    op0=ALU.add, op1=ALU.add,
                                )
                            if last_sample:
                                deng = (nc.sync, nc.scalar, nc.gpsimd)[
                                    (2 * cm + h0 // 512) % 3]
                            else:
                                deng = nc.sync
                            deng.dma_start(
                                out=out_d[b, cm * P:(cm + 1) * P, h0:h1],
                                in_=out_t[:, h0:h1])

                # tail: stats(2) on iter-0 (x[2] lands mid-iter-0), then
                # affine(b+2) + apply(b+2); the bc matmul runs after PV(b).
                if b + 2 < NB:
                    h_cur = h_next
                    if b == 0:
                        mv_next, _ = gn_stats(b + 2, after=last_apply)
                    gn_affine(b + 2, mv_next)
                    h_next, last_apply = apply_h(b + 2, after=last_tmp)
                else:
                    h_cur = h_next

    nc.compile()
    return nc


def prep_inputs(inputs):
    """Host-side prep: per-core in_maps with fused, pre-laid-out weights."""
    e4 = ml_dtypes.float8_e4m3
    x = np.ascontiguousarray(np.asarray(inputs["x"], dtype=np.float32)).reshape(
        B_FULL, C, HW
    )

    Wq = np.asarray(inputs["Wq"], dtype=np.float64)
    Wk = np.asarray(inputs["Wk"], dtype=np.float64)
    Wv = np.asarray(inputs["Wv"], dtype=np.float64)
    Wt = np.asarray(inputs["Wt"], dtype=np.float64)
    A = (WSC * (Wq @ Wk.T)).astype(np.float32)   # scores = h^T A h / 8
    Bm = (WSC * (Wv @ Wt)).astype(np.float32)    # out = (B^T h)^T P / 8

    def wprep(w):
        # [C, C] -> [P, NT, C]  (lhsT slices w[kc*128+p, d])
        return np.ascontiguousarray(
            np.asarray(w, dtype=np.float32).reshape(NT, P, C).transpose(1, 0, 2)
        ).astype(e4)

    def cols(v):
        # [C] -> [P, NT]
        return np.ascontiguousarray(
            np.asarray(v, dtype=np.float32).reshape(NT, P).T
        )

    gab = np.stack([cols(inputs["gn_scale"]), cols(inputs["gn_bias"])], axis=1)
    gg = np.zeros((P, P), np.float32)
    for p in range(P):
        gg[p, (p // GS) * GS:(p // GS + 1) * GS] = 1.0 / GS

    btp = (np.asarray(inputs["bt"], dtype=np.float64)
           + Wt.T @ np.asarray(inputs["bv"], dtype=np.float64)).astype(np.float32)

    shared = {
        "wqc": wprep(A), "wvt": wprep(Bm),
        "gn_ab": np.ascontiguousarray(gab), "gg": gg,
    }
    if np.any(btp != 0):
        shared["btp"] = cols(btp)
    in_maps = []
    for c_id in range(N_CORES):
        m = dict(shared)
        m["x"] = np.ascontiguousarray(x[c_id * NB:(c_id + 1) * NB])
        in_maps.append(m)
    return in_maps


_NC_CACHE = {}


def get_nc(zero_out_bias=True):
    key = (zero_out_bias,)
    if key not in _NC_CACHE:
        _NC_CACHE[key] = build_nc(zero_out_bias=zero_out_bias)
    return _NC_CACHE[key]


def _kernel_numpy(inputs):
    """Pure-numpy fallback, only for nonzero bq/bk (never hit by this
    problem's inputs -- setup_inputs() uses zero biases)."""
    x = np.asarray(inputs["x"], dtype=np.float64)
    B, C_, H_, W_ = x.shape
    g = x.reshape(B, NGROUPS, C_ // NGROUPS, H_, W_)
    mean = g.mean(axis=(2, 3, 4), keepdims=True)
    var = g.var(axis=(2, 3, 4), keepdims=True)
    hn = ((g - mean) / np.sqrt(var + EPS)).reshape(B, C_, H_, W_)
    hn = hn * np.asarray(inputs["gn_scale"], np.float64)[None, :, None, None] \
        + np.asarray(inputs["gn_bias"], np.float64)[None, :, None, None]

    def nin(h, Wm, bm):
        return np.einsum("bchw,cd->bdhw", h, np.asarray(Wm, np.float64)) \
            + np.asarray(bm, np.float64)[None, :, None, None]

    q = nin(hn, inputs["Wq"], inputs["bq"])
    k = nin(hn, inputs["Wk"], inputs["bk"])
    v = nin(hn, inputs["Wv"], inputs["bv"])
    w = np.einsum("bchw,bcij->bhwij", q, k) * (C_ ** -0.5)
    w = w.reshape(B, H_, W_, H_ * W_)
    w = np.exp(w - w.max(axis=-1, keepdims=True))
    w = (w / w.sum(axis=-1, keepdims=True)).reshape(B, H_, W_, H_, W_)
    hh = np.einsum("bhwij,bcij->bchw", w, v)
    hh = nin(hh, inputs["Wt"], inputs["bt"])
    return (hh + x).astype(np.float32)


def run(inputs, trace=False):
    from concourse.bass_utils import run_bass_kernel_spmd

    if not (np.all(np.asarray(inputs["bq"]) == 0)
            and np.all(np.asarray(inputs["bk"]) == 0)):
        return _kernel_numpy(inputs), None

    in_maps = prep_inputs(inputs)
    nc = get_nc(zero_out_bias="btp" not in in_maps[0])
    res = run_bass_kernel_spmd(
        nc, in_maps, core_ids=list(range(N_CORES)), trace=trace
    )
    out = np.concatenate([np.asarray(r["out"]) for r in res.results], axis=0)
    return out.reshape(B_FULL, C, H, W), res


def kernel(**inputs):
    out, _ = run(inputs, trace=False)
    return out


# revision 46
# speedup vs baseline: 1.1755x; 1.0104x over previous
"""AttnBlock (GroupNorm -> QKV 1x1 -> full attention over 1024 tokens -> out-proj
+ residual) for x [32, 512, 32, 32] f32, distributed data-parallel over 8
NeuronCores (4 samples per core, weights replicated).

Per-core single-NC Bass/Tile kernel, v2: weight-fusion + all-fp8 DoubleRow.

Algebraic restructuring (host-side, exact in f32):
  - scores  = (Wq h)^T (Wk h) = h^T A h with A = Wq @ Wk.T  -> ONE fused
    "qc" projection (qc = A^T h) replaces the separate Q and K projections.
  - out-proj fused into V: vt' = (Wv @ Wt)^T h gives
    out = vt'^T P_norm  directly, so the PV matmul's PSUM already holds the
    final pre-residual activation -- the separate out-projection disappears.
  - A and B=Wv@Wt are pre-scaled by 8 on the host so their entries clear the
    fp8e4 subnormal range; the 8x cancels via exp-scale (CINV/8) on the
    score side and via rep = 1/(8*den) on the PV side.

Per-sample PE work drops 82944 -> ~50200 col-cycles (scores 16384 + PV-DR
16384 + qc 8192 + vt' 8192 + den 1024): every matmul issues moving columns at
1/cycle regardless of dtype, so fp8 DoubleRow's 2x comes purely from halving
the pass count (contraction 256 rows/pass).

  - est = exp(s*c - 4.5) stored fp8e5 (e5m2: scores empirically reach 13.8
    sigma, far past fp8e4's e^11.7 dynamic range; e5m2 spans it easily and
    softmax normalization cancels most of its 2-mantissa-bit noise -- host
    emulation: 0.0078 rel err vs 0.0068 for bf16 est). The offset cancels
    exactly in softmax. PV runs fp8 DoubleRow (e4m3 vt x e5m2 est) over
    jm-pair passes.

Hardware scheduling lessons baked in (each measured on traces):
  - ACT and Sync have exec-queue depth 0 (strict head-of-line blocking):
    never give them an op that waits on another engine ahead of
    latency-critical work. DVE (depth 8) and PE absorb such inversions.
  - Each DMA transfer is serviced by ~one engine at ~21 B/ns and needs
    2KB-contiguous rows + full 128 partitions for full rate; parallelism
    = in-flight transfer count. x loads are 8x256KB transfers, gated by
    semaphores on GN progress so the wire stays exclusive to the next
    needed tensor.
  - PSUM pool rotations create hidden cross-engine serialization: the
    den-reduce psums get a dedicated pool (rsp) so qc(b+1) never chains
    behind recip(b); tmpp has 4 bufs so the GpSimd residual adds stay off
    the critical path into scores(b+1).
  - softmax denominator: DVE pairwise-add tree over the 8 fp8 exp tiles
    (bf16 out, 2x DVE rate), one 8.0-matrix bf16 matmul reduces partitions
    and replicates 8*den; rep = 1/(8 den) via reciprocal_approx_fast.
  - residual: tmp = PV_psum * rep on DVE, out = tmp + x on GpSimd (idle
    otherwise; DVE for the last sample to shorten the tail), DMA per half.
  - engine balance per sample: PE ~20.9us, DVE ~19.8us (GN stats/apply, den
    tree, recip, tmp), ACT ~17.4us (exp + qc/vt' psum->sbuf fp8 copies),
    GpSimd ~9us (residual adds + out DMA issue).
  - prologue: x[0] owns all three DMA rings before weights/x[1..3] queue up;
    GN(0) stats start per-tile as x[0] tiles land.
"""

import os
import sys

import numpy as np

sys.path.insert(0, "/opt/trn_rl_repo")

import ml_dtypes  # noqa: E402

import concourse.bass as bass  # noqa: E402
import concourse.tile as tile  # noqa: E402
from concourse import bacc, mybir  # noqa: E402

P = 128
B_FULL, C, H, W = 32, 512, 32, 32
HW = H * W            # 1024 tokens
N_CORES = 8
NB = B_FULL // N_CORES  # 4 samples per core
NT = C // P           # 4 channel tiles
NP = NT // 2          # 2 DoubleRow channel-tile pairs
NJ = HW // P          # 8 token tiles
NJP = NJ // 2         # 4 DoubleRow token-tile pairs
NGROUPS = 32
GS = C // NGROUPS     # 16 channels per group
EPS = 1e-6
CINV = float(C) ** -0.5
WSC = 8.0             # fused weights pre-scaled by 8 (fp8 subnormal guard)
EOFF = -4.5           # exp offset: est = exp(s*c - 4.5), cancels in softmax

f32 = mybir.dt.float32
bf16 = mybir.dt.bfloat16
f8 = mybir.dt.float8e4
f8e5 = mybir.dt.float8e5
ALU = mybir.AluOpType
ACT = mybir.ActivationFunctionType
DR = mybir.MatmulPerfMode.DoubleRow


def build_nc(zero_out_bias=True):
    """Build the single-core Bass graph (SPMD: same graph on all 8 cores).

    zero_out_bias: fused output bias bt' = bt + Wt^T bv is all-zero (true for
    this problem), so the residual add drops the bias column.
    """
    nc = bacc.Bacc("TRN2", target_bir_lowering=False, debug=False)

    x_d = nc.dram_tensor("x", [NB, C, HW], f32, kind="ExternalInput")
    wqc_d = nc.dram_tensor("wqc", [P, NT, C], f8, kind="ExternalInput")
    wvt_d = nc.dram_tensor("wvt", [P, NT, C], f8, kind="ExternalInput")
    # gn affine columns: [:, 0, :]=gamma, [:, 1, :]=beta
    gab_d = nc.dram_tensor("gn_ab", [P, 2, NT], f32, kind="ExternalInput")
    # block-diagonal group-average matrix: GG[k,p] = 1/16 iff k//16 == p//16
    gg_d = nc.dram_tensor("gg", [P, P], f32, kind="ExternalInput")
    if not zero_out_bias:
        btp_d = nc.dram_tensor("btp", [P, NT], f32, kind="ExternalInput")
    out_d = nc.dram_tensor("out", [NB, C, HW], f32, kind="ExternalOutput")

    with tile.TileContext(nc) as tc:
        with (
            tc.tile_pool(name="consts", bufs=1) as consts,
            tc.tile_pool(name="hp", bufs=2) as hp,
            tc.tile_pool(name="qcp", bufs=2) as qcp,
            tc.tile_pool(name="vtp", bufs=2) as vtp,
            tc.tile_pool(name="est", bufs=1) as estp,
            tc.tile_pool(name="sump", bufs=1) as sump,
            tc.tile_pool(name="repp", bufs=1) as repp,
            # tmpp=4: with only 2 bufs, tmp(cm3)'s alloc waits the GpSimd
            # residual-add of cm1, putting the slow GpSimd chain on the
            # critical path into scores(b+1) via the psb rotation (measured
            # ~5us/iter stall).
            tc.tile_pool(name="tmpp", bufs=4) as tmpp,
            tc.tile_pool(name="outp", bufs=4) as outp,
            tc.tile_pool(name="small", bufs=2) as small,
            # psb 2x[P,1024] (4) + psq 2x[P,512] (2) + rsp 2x[P,512] (2)
            # = all 8 banks. The den-reduce psums get their OWN pool (rsp):
            # inside the psq rotation, qc(b+1)'s first psum alloc waits on
            # recip(b), serializing the whole next-sample projection phase
            # behind the exp->gpsimd-tree->den-red->recip latency chain
            # (~10us/iter, measured).
            tc.tile_pool(name="psb", bufs=2, space="PSUM") as psb,
            tc.tile_pool(name="psq", bufs=2, space="PSUM") as psq,
            tc.tile_pool(name="rsp", bufs=2, space="PSUM") as rsp,
        ):
            x_sb = consts.tile([P, NB, NT, HW], f32, tag="x")
            wqc_sb = consts.tile([P, NT, C], f8, tag="wqc")
            wvt_sb = consts.tile([P, NT, C], f8, tag="wvt")
            gab_sb = consts.tile([P, 2, NT], f32, tag="gab")
            gg_sb = consts.tile([P, P], f32, tag="gg")
            if not zero_out_bias:
                btp_sb = consts.tile([P, NT], f32, tag="btp")

            # gg+gab first on gpsimd (tiny, needed by affine(0)); then x[0]
            # halves (2KB contiguous per partition row) own ALL THREE rings.
            # DMA engines pull from all queued transfers CONCURRENTLY (not
            # ring-order), so x[1..3] loads are semaphore-GATED behind GN
            # progress below -- otherwise they steal ~2/3 of the wire and
            # x[0] (which gates the whole GN(0)->qc(0) prologue chain) lands
            # ~7us late (measured).
            # Each DMA transfer is serviced by ~one engine at ~21 B/ns, so
            # wire parallelism = in-flight transfer count. Transfers must
            # keep 2KB-contiguous rows and FULL 128 partitions (64-partition
            # transfers run at half rate -- measured), so a 2MB sample maxes
            # out at 8 transfers = ~12us. x[1..3] are semaphore-gated behind
            # GN progress so they don't steal the wire from x[0].
            nc.gpsimd.dma_start(out=gg_sb[:, :], in_=gg_d[:, :])
            nc.gpsimd.dma_start(out=gab_sb[:, :, :], in_=gab_d[:, :, :])
            engs = (nc.sync, nc.scalar, nc.gpsimd)
            qi = 0
            for t in range(NT):
                for h0 in (0, 512):
                    engs[qi % 3].dma_start(
                        out=x_sb[:, 0, t, h0:h0 + 512],
                        in_=x_d[0, t * P:(t + 1) * P, h0:h0 + 512])
                    qi += 1
            # weights in 2 transfers each, right behind x[0]
            for t2 in (0, 2):
                nc.sync.dma_start(out=wqc_sb[:, t2:t2 + 2, :],
                                  in_=wqc_d[:, t2:t2 + 2, :])
                nc.scalar.dma_start(out=wvt_sb[:, t2:t2 + 2, :],
                                    in_=wvt_d[:, t2:t2 + 2, :])
            if not zero_out_bias:
                nc.gpsimd.dma_start(out=btp_sb[:, :], in_=btp_d[:, :])

            def load_x(b, eng_list, after):
                """Issue sample b's x as 8 half-tile transfers (full rate)
                round-robined over eng_list, gated behind `after`."""
                qi = 0
                for t in range(NT):
                    for h0 in (0, 512):
                        eng = eng_list[qi % len(eng_list)]
                        qi += 1
                        dd = eng.dma_start(
                            out=x_sb[:, b, t, h0:h0 + 512],
                            in_=x_d[b, t * P:(t + 1) * P, h0:h0 + 512])
                        if after is not None:
                            tile.add_dep_helper(dd.ins, after.ins, sync=True,
                                                reason="x load gated on GN progress")

            # 8.0-matrix: reduces den partials across partitions AND folds the
            # 8x fused-weight scale (rep = 1/(8*den))
            ones_sb = consts.tile([P, P], bf16, tag="ones")
            nc.vector.memset(ones_sb[:, :], WSC)
            magic_sb = consts.tile([P, NT], mybir.dt.int32, tag="magic")
            nc.vector.memset(magic_sb[:, :], 0x5F3759DF)
            # per-partition exp-offset column (bias operand for the Exp calls)
            eoff_sb = consts.tile([P, 1], f32, tag="eoff")
            nc.vector.memset(eoff_sb[:, :], EOFF)
            # dummy Exp: pulls the ACT Exp-table load into the idle prologue
            expwarm = small.tile([P, 1], f32, tag="expwarm")
            nc.scalar.activation(out=expwarm[:, :], in_=ones_sb[:, 0:1],
                                 func=ACT.Exp, scale=CINV / WSC,
                                 bias=eoff_sb[:, 0:1])

            a_all = consts.tile([P, NB, NT], f32, tag="a_all")
            b_all = consts.tile([P, NB, NT], f32, tag="b_all")

            # PE warm-up on gg (earliest-arriving tensor): releases the HAM
            # clock-gate before the real stream.
            warm_ps = psq.tile([P, 512], f32, tag="qkv")
            for w in range(5):
                nc.tensor.matmul(
                    warm_ps[:, 0:128], gg_sb[:, 0:128], gg_sb[:, 0:128],
                    start=(w == 0), stop=(w == 4),
                )

            def gn_stats(b, after=None, mv=None, ts=0, te=NT):
                """bn stats -> per-channel (mean, Ex2) in mv[:, ts:te, :].
                GN groups (16ch) never span 128-channel tiles, so the whole
                GN chain is per-tile independent -- the prologue exploits
                this with 2-tile batches that chase the x DMAs."""
                if mv is None:
                    mv = small.tile([P, NT, 2], f32, tag="mv")
                nsub = 2
                step = HW // nsub
                first = None
                for t in range(ts, te):
                    st6 = small.tile([P, nsub, 6], f32, tag="st6")
                    for q in range(nsub):
                        iq = nc.vector.bn_stats(
                            out=st6[:, q, :],
                            in_=x_sb[:, b, t, q * step:(q + 1) * step])
                        if first is None:
                            first = iq
                        if after is not None:
                            tile.add_dep_helper(iq.ins, after.ins, sync=False,
                                                reason="gn stats after prev apply")
                    nc.vector.bn_aggr(out=mv[:, t, :], in_=st6[:, :, :])
                msq = small.tile([P, NT], f32, tag="msq")
                nc.vector.tensor_mul(msq[:, ts:te], mv[:, ts:te, 0],
                                     mv[:, ts:te, 0])
                nc.vector.tensor_add(mv[:, ts:te, 1], mv[:, ts:te, 1],
                                     msq[:, ts:te])
                return mv, first

            def gn_affine(b, mv, ts=0, te=NT):
                """fused group-avg+broadcast matmul, then per-channel A/B
                for tiles [ts, te)."""
                nw = (te - ts) * 2
                bc_ps = psq.tile([P, 512], f32, tag="qkv")
                nc.tensor.matmul(bc_ps[:, :nw], gg_sb[:, :], mv[:, ts:te, :],
                                 start=True, stop=True)
                bc = small.tile([P, NT, 2], f32, tag="bcs")
                nc.vector.tensor_copy(bc[:, ts:te, :], bc_ps[:, 0:nw])
                vb = small.tile([P, NT], f32, tag="vb")
                nc.vector.tensor_mul(vb[:, ts:te], bc[:, ts:te, 0], bc[:, ts:te, 0])
                nc.vector.tensor_sub(vb[:, ts:te], bc[:, ts:te, 1], vb[:, ts:te])
                nc.vector.tensor_scalar_add(vb[:, ts:te], vb[:, ts:te], EPS)
                # rstd = rsqrt(var+eps): fast-inverse-sqrt + 1 Newton step
                # (all-DVE: keeps Sqrt off ACT so it never evicts Exp)
                ii = small.tile([P, NT], mybir.dt.int32, tag="ii")
                nc.vector.tensor_scalar(
                    out=ii[:, ts:te], in0=vb.bitcast(mybir.dt.int32)[:, ts:te],
                    scalar1=1, scalar2=None, op0=ALU.arith_shift_right)
                nc.vector.tensor_tensor(ii[:, ts:te], magic_sb[:, ts:te],
                                        ii[:, ts:te], op=ALU.subtract)
                y0 = ii.bitcast(f32)
                yt = small.tile([P, NT], f32, tag="yt")
                y1 = small.tile([P, NT], f32, tag="y1")
                nc.vector.tensor_mul(yt[:, ts:te], vb[:, ts:te], y0[:, ts:te])
                nc.vector.tensor_mul(yt[:, ts:te], yt[:, ts:te], y0[:, ts:te])
                nc.vector.tensor_scalar(out=yt[:, ts:te], in0=yt[:, ts:te],
                                        scalar1=-0.5, scalar2=1.5,
                                        op0=ALU.mult, op1=ALU.add)
                nc.vector.tensor_mul(y1[:, ts:te], y0[:, ts:te], yt[:, ts:te])
                tmp = small.tile([P, NT], f32, tag="tmpab")
                nc.vector.tensor_mul(a_all[:, b, ts:te], y1[:, ts:te],
                                     gab_sb[:, 0, ts:te])
                nc.vector.tensor_mul(tmp[:, ts:te], bc[:, ts:te, 0],
                                     a_all[:, b, ts:te])
                nc.vector.tensor_sub(b_all[:, b, ts:te], gab_sb[:, 1, ts:te],
                                     tmp[:, ts:te])

            def apply_h(b, after=None, h=None, ts=0, te=NT):
                """h = x*A + B (fp8), all-DVE. (NOT on ACT: the Activation
                engine has exec-queue depth 0, so an apply waiting on the
                DVE affine chain head-blocks the whole exp stream -- cost
                ~5us/iter, measured. DVE's depth-8 queue absorbs it.)"""
                if h is None:
                    h = hp.tile([P, NT, HW], f8, tag="h")
                last = None
                for t in range(ts, te):
                    last = nc.vector.tensor_scalar(
                        out=h[:, t, :], in0=x_sb[:, b, t, :],
                        scalar1=a_all[:, b, t:t + 1],
                        scalar2=b_all[:, b, t:t + 1],
                        op0=ALU.mult, op1=ALU.add,
                    )
                    if after is not None:
                        tile.add_dep_helper(last.ins, after.ins, sync=False,
                                            reason="apply after prev tmp")
                return h, last

            def gn_full_batched(b, after=None):
                """prologue GN in two 2-tile batches so apply(t0,t1) lands
                as soon as those x tiles arrive (qc's first DR pass only
                needs h tiles 0-1)."""
                mv = small.tile([P, NT, 2], f32, tag="mv")
                h = hp.tile([P, NT, HW], f8, tag="h")
                mv, first = gn_stats(b, after=after, mv=mv, ts=0, te=2)
                gn_affine(b, mv, 0, 2)
                _, last = apply_h(b, h=h, ts=0, te=2)
                gn_stats(b, mv=mv, ts=2, te=NT)
                gn_affine(b, mv, 2, NT)
                _, last = apply_h(b, h=h, ts=2, te=NT)
                return h, last, first

            def qc_proj(b, h_sb):
                """qc = A^T h (fp8 [d, hw], 8x-scaled); psum->sbuf on ACT."""
                qc_sb = qcp.tile([P, NT, HW], f8, tag="qc")
                for dm in range(NT):
                    ps0 = psq.tile([P, 512], f32, tag="qkv")
                    ps1 = psq.tile([P, 512], f32, tag="qkv")
                    for kp in range(NP):
                        lhsT = wqc_sb[:, 2 * kp:2 * kp + 2, dm * P:(dm + 1) * P]
                        st, sp = (kp == 0), (kp == NP - 1)
                        nc.tensor.matmul(ps0[:, :], lhsT,
                                         h_sb[:, 2 * kp:2 * kp + 2, 0:512],
                                         start=st, stop=sp, perf_mode=DR)
                        nc.tensor.matmul(ps1[:, :], lhsT,
                                         h_sb[:, 2 * kp:2 * kp + 2, 512:1024],
                                         start=st, stop=sp, perf_mode=DR)
                    nc.scalar.copy(qc_sb[:, dm, 0:512], ps0[:, :])
                    nc.scalar.copy(qc_sb[:, dm, 512:1024], ps1[:, :])
                return qc_sb

            def vt_proj(b, h_sb, split_copies=False):
                """vt' = (Wv Wt)^T h, transposed [hw, d] fp8 (8x-scaled);
                psum->sbuf on ACT (alternating with DVE for the deferred
                vt'(0), whose copies would otherwise stack up behind qc(1)'s
                in iteration 0's ACT queue)."""
                vt_sb = vtp.tile([P, NJ, C], f8, tag="vt")
                for jm in range(NJ):
                    ps = psq.tile([P, 512], f32, tag="qkv")
                    for kp in range(NP):
                        nc.tensor.matmul(
                            ps[:, :],
                            h_sb[:, 2 * kp:2 * kp + 2, jm * P:(jm + 1) * P],
                            wvt_sb[:, 2 * kp:2 * kp + 2, :],
                            start=(kp == 0), stop=(kp == NP - 1), perf_mode=DR,
                        )
                    if split_copies and jm % 2 == 0:
                        nc.vector.tensor_copy(vt_sb[:, jm, :], ps[:, :])
                    else:
                        nc.scalar.copy(vt_sb[:, jm, :], ps[:, :])
                return vt_sb

            # ---------------- prologue ----------------
            # vt'(0) is NOT issued here: it would delay scores(0) by 3.4us of
            # PE time; it slots into iteration 0 between qc(1) and vt'(1),
            # where its ACT copies run after exp(0) drains.
            # x[1] loads fire once GN(0) is underway (x[0] fully landed soon
            # after); x[2]/x[3] once GN(1) starts: keeps the wire exclusive
            # to whatever the pipeline needs next. (GN is NOT tile-batched:
            # the affine chain is small-op-overhead-dominated, and doubling
            # it costs more than the earlier apply saves -- measured.)
            mv0, stats0_first = gn_stats(0)
            load_x(1, (nc.sync, nc.scalar), stats0_first)
            gn_affine(0, mv0)
            h0_sb, last_apply = apply_h(0)

            qc_next = qc_proj(0, h0_sb)

            mv1, stats1_first = gn_stats(1, after=last_apply)
            load_x(2, (nc.gpsimd, nc.sync), stats1_first)
            load_x(3, (nc.sync, nc.gpsimd), stats1_first)
            gn_affine(1, mv1)
            h_next, last_apply = apply_h(1)

            vt_next = None
            h_cur = h0_sb
            last_tmp = None

            # ---------------- main loop ----------------
            mv_next = None
            for b in range(NB):
                qc_sb = qc_next
                vt_sb = vt_next

                # iter>=1: GN stats for b+2 at the DVE-queue HEAD -- its x
                # landed long ago, it has no cross-engine deps, and it fills
                # the DVE idle while exp(b) produces the den-tree inputs.
                if b >= 1 and b + 2 < NB:
                    mv_next, _ = gn_stats(b + 2, after=last_apply)

                # ---- scores transposed (ST[j,i] = sum_d h[d,j] qc[d,i])
                #      + exp (ACT, fp8 out with -2 offset) ----
                est_sb = estp.tile([P, NJ, HW], f8e5, tag="est")
                for jm in range(NJ):
                    st_ps = psb.tile([P, HW], f32, tag="big")
                    for kp in range(NP):
                        lhsT = h_cur[:, 2 * kp:2 * kp + 2, jm * P:(jm + 1) * P]
                        st, sp = (kp == 0), (kp == NP - 1)
                        nc.tensor.matmul(st_ps[:, 0:512], lhsT,
                                         qc_sb[:, 2 * kp:2 * kp + 2, 0:512],
                                         start=st, stop=sp, perf_mode=DR)
                        nc.tensor.matmul(st_ps[:, 512:1024], lhsT,
                                         qc_sb[:, 2 * kp:2 * kp + 2, 512:1024],
                                         start=st, stop=sp, perf_mode=DR)
                    nc.scalar.activation(
                        out=est_sb[:, jm, :], in_=st_ps[:, :], func=ACT.Exp,
                        scale=CINV / WSC, bias=eoff_sb[:, 0:1],
                    )

                # ---- softmax denominator tree (bf16 out): level-1 split
                # 2 adds DVE / 2 adds GpSimd (idle early-iteration) ----
                s0 = sump.tile([P, HW], bf16, tag="s0")
                s1 = sump.tile([P, HW], bf16, tag="s1")
                s2 = sump.tile([P, HW], bf16, tag="s2")
                s3 = sump.tile([P, HW], bf16, tag="s3")
                nc.vector.tensor_add(s0[:, :], est_sb[:, 0, :], est_sb[:, 1, :])
                nc.gpsimd.tensor_add(s1[:, :], est_sb[:, 2, :], est_sb[:, 3, :])
                nc.vector.tensor_add(s2[:, :], est_sb[:, 4, :], est_sb[:, 5, :])
                nc.gpsimd.tensor_add(s3[:, :], est_sb[:, 6, :], est_sb[:, 7, :])
                nc.vector.tensor_add(s0[:, :], s0[:, :], s1[:, :])
                nc.vector.tensor_add(s2[:, :], s2[:, :], s3[:, :])
                nc.vector.tensor_add(s0[:, :], s0[:, :], s2[:, :])

                # ---- qc, vt' for sample b+1 fill TensorE while ACT exps ----
                if b + 1 < NB:
                    qc_next = qc_proj(b + 1, h_next)
                if b == 0:
                    vt_sb = vt_proj(0, h0_sb)
                if b + 1 < NB:
                    vt_next = vt_proj(b + 1, h_next)

                # ---- PV fp8 DoubleRow over jm pairs: psum IS the final
                #      pre-residual activation (out-proj fused into vt').
                #      The den partition-reduce slots in AFTER PV cm0: rep is
                #      first needed by tmp(cm0), so this buys the DVE den
                #      tree ~2.3us of extra slack before the PE blocks on it.
                last_tmp = None
                rep = repp.tile([P, HW], f32, tag="rep")
                last_sample = b == NB - 1

                def den_recip():
                    rs0 = rsp.tile([P, 512], f32, tag="rs")
                    rs1 = rsp.tile([P, 512], f32, tag="rs")
                    nc.tensor.matmul(rs0[:, :], ones_sb[:, :], s0[:, 0:512],
                                     start=True, stop=True)
                    nc.tensor.matmul(rs1[:, :], ones_sb[:, :],
                                     s0[:, 512:1024], start=True, stop=True)
                    nc.vector.reciprocal_approx_fast(out=rep[:, 0:512],
                                                     in_=rs0[:, :])
                    nc.vector.reciprocal_approx_fast(out=rep[:, 512:1024],
                                                     in_=rs1[:, :])

                for cm in range(NT):
                    o_ps = psb.tile([P, HW], f32, tag="big")
                    for t in range(NJP):
                        lhsT = vt_sb[:, 2 * t:2 * t + 2, cm * P:(cm + 1) * P]
                        st, sp = (t == 0), (t == NJP - 1)
                        nc.tensor.matmul(o_ps[:, 0:512], lhsT,
                                         est_sb[:, 2 * t:2 * t + 2, 0:512],
                                         start=st, stop=sp, perf_mode=DR)
                        nc.tensor.matmul(o_ps[:, 512:1024], lhsT,
                                         est_sb[:, 2 * t:2 * t + 2, 512:1024],
                                         start=st, stop=sp, perf_mode=DR)
                    if cm == 0:
                        # den partition-reduce after PV cm0 (see note above)
                        den_recip()
                    # tmp = psum * rep on DVE; residual add on GpSimd
                    # (DVE for the last sample: shorter tail); DMA issues on
                    # the idle Sync engine (~600ns engine time each)
                    tmv = tmpp.tile([P, HW], f32, tag="tmp")
                    last_tmp = nc.vector.tensor_mul(tmv[:, :], o_ps[:, :],
                                                    rep[:, :])
                    out_t = outp.tile([P, HW], f32, tag="out")
                    for hh in (0, 512):
                        if zero_out_bias:
                            eng = nc.vector if last_sample else nc.gpsimd
                            eng.tensor_add(out_t[:, hh:hh + 512],
                                           tmv[:, hh:hh + 512],
                                           x_sb[:, b, cm, hh:hh + 512])
                        else:
                            nc.vector.scalar_tensor_tensor(
                                out=out_t[:, hh:hh + 512],
                                in0=tmv[:, hh:hh + 512],
                                scalar=btp_sb[:, cm:cm + 1],
                                in1=x_sb[:, b, cm, hh:hh + 512],
                                op0=ALU.add, op1=ALU.add,
                            )
                        deng = nc.scalar if (last_sample and hh) else nc.sync
                        deng.dma_start(
                            out=out_d[b, cm * P:(cm + 1) * P, hh:hh + 512],
                            in_=out_t[:, hh:hh + 512])

                # tail: stats(2) on iter-0 (x[2] lands mid-iter-0), then
                # affine(b+2) + apply(b+2); the bc matmul runs after PV(b).
                if b + 2 < NB:
                    h_cur = h_next
                    if b == 0:
                        mv_next, _ = gn_stats(b + 2, after=last_apply)
                    gn_affine(b + 2, mv_next)
                    h_next, last_apply = apply_h(b + 2, after=last_tmp)
                else:
                    h_cur = h_next

    nc.compile()
    return nc


def prep_inputs(inputs):
    """Host-side prep: per-core in_maps with fused, pre-laid-out weights."""
    e4 = ml_dtypes.float8_e4m3
    x = np.ascontiguousarray(np.asarray(inputs["x"], dtype=np.float32)).reshape(
        B_FULL, C, HW
    )

    Wq = np.asarray(inputs["Wq"], dtype=np.float64)
    Wk = np.asarray(inputs["Wk"], dtype=np.float64)
    Wv = np.asarray(inputs["Wv"], dtype=np.float64)
    Wt = np.asarray(inputs["Wt"], dtype=np.float64)
    A = (WSC * (Wq @ Wk.T)).astype(np.float32)   # scores = h^T A h / 8
    Bm = (WSC * (Wv @ Wt)).astype(np.float32)    # out = (B^T h)^T P / 8

    def wprep(w):
        # [C, C] -> [P, NT, C]  (lhsT slices w[kc*128+p, d])
        return np.ascontiguousarray(
            np.asarray(w, dtype=np.float32).reshape(NT, P, C).transpose(1, 0, 2)
        ).astype(e4)

    def cols(v):
        # [C] -> [P, NT]
        return np.ascontiguousarray(
            np.asarray(v, dtype=np.float32).reshape(NT, P).T
        )

    gab = np.stack([cols(inputs["gn_scale"]), cols(inputs["gn_bias"])], axis=1)
    gg = np.zeros((P, P), np.float32)
    for p in range(P):
        gg[p, (p // GS) * GS:(p // GS + 1) * GS] = 1.0 / GS

    btp = (np.asarray(inputs["bt"], dtype=np.float64)
           + Wt.T @ np.asarray(inputs["bv"], dtype=np.float64)).astype(np.float32)

    shared = {
        "wqc": wprep(A), "wvt": wprep(Bm),
        "gn_ab": np.ascontiguousarray(gab), "gg": gg,
    }
    if np.any(btp != 0):
        shared["btp"] = cols(btp)
    in_maps = []
    for c_id in range(N_CORES):
        m = dict(shared)
        m["x"] = np.ascontiguousarray(x[c_id * NB:(c_id + 1) * NB])
        in_maps.append(m)
    return in_maps


_NC_CACHE = {}


def get_nc(zero_out_bias=True):
    key = (zero_out_bias,)
    if key not in _NC_CACHE:
        _NC_CACHE[key] = build_nc(zero_out_bias=zero_out_bias)
    return _NC_CACHE[key]


def _kernel_numpy(inputs):
    """Pure-numpy fallback, only for nonzero bq/bk (never hit by this
    problem's inputs -- setup_inputs() uses zero biases)."""
    x = np.asarray(inputs["x"], dtype=np.float64)
    B, C_, H_, W_ = x.shape
    g = x.reshape(B, NGROUPS, C_ // NGROUPS, H_, W_)
    mean = g.mean(axis=(2, 3, 4), keepdims=True)
    var = g.var(axis=(2, 3, 4), keepdims=True)
    hn = ((g - mean) / np.sqrt(var + EPS)).reshape(B, C_, H_, W_)
    hn = hn * np.asarray(inputs["gn_scale"], np.float64)[None, :, None, None] \
        + np.asarray(inputs["gn_bias"], np.float64)[None, :, None, None]

    def nin(h, Wm, bm):
        return np.einsum("bchw,cd->bdhw", h, np.asarray(Wm, np.float64)) \
            + np.asarray(bm, np.float64)[None, :, None, None]

    q = nin(hn, inputs["Wq"], inputs["bq"])
    k = nin(hn, inputs["Wk"], inputs["bk"])
    v = nin(hn, inputs["Wv"], inputs["bv"])
    w = np.einsum("bchw,bcij->bhwij", q, k) * (C_ ** -0.5)
    w = w.reshape(B, H_, W_, H_ * W_)
    w = np.exp(w - w.max(axis=-1, keepdims=True))
    w = (w / w.sum(axis=-1, keepdims=True)).reshape(B, H_, W_, H_, W_)
    hh = np.einsum("bhwij,bcij->bchw", w, v)
    hh = nin(hh, inputs["Wt"], inputs["bt"])
    return (hh + x).astype(np.float32)


def run(inputs, trace=False):
    from concourse.bass_utils import run_bass_kernel_spmd

    if not (np.all(np.asarray(inputs["bq"]) == 0)
            and np.all(np.asarray(inputs["bk"]) == 0)):
        return _kernel_numpy(inputs), None

    in_maps = prep_inputs(inputs)
    nc = get_nc(zero_out_bias="btp" not in in_maps[0])
    res = run_bass_kernel_spmd(
        nc, in_maps, core_ids=list(range(N_CORES)), trace=trace
    )
    out = np.concatenate([np.asarray(r["out"]) for r in res.results], axis=0)
    return out.reshape(B_FULL, C, H, W), res


def kernel(**inputs):
    out, _ = run(inputs, trace=False)
    return out


# revision 48
# speedup vs baseline: 1.1956x; 1.0171x over previous
"""AttnBlock (GroupNorm -> QKV 1x1 -> full attention over 1024 tokens -> out-proj
+ residual) for x [32, 512, 32, 32] f32, distributed data-parallel over 8
NeuronCores (4 samples per core, weights replicated).

Per-core single-NC Bass/Tile kernel, v2: weight-fusion + all-fp8 DoubleRow.

Algebraic restructuring (host-side, exact in f32):
  - scores  = (Wq h)^T (Wk h) = h^T A h with A = Wq @ Wk.T  -> ONE fused
    "qc" projection (qc = A^T h) replaces the separate Q and K projections.
  - out-proj fused into V: vt' = (Wv @ Wt)^T h gives
    out = vt'^T P_norm  directly, so the PV matmul's PSUM already holds the
    final pre-residual activation -- the separate out-projection disappears.
  - A and B=Wv@Wt are pre-scaled by 8 on the host so their entries clear the
    fp8e4 subnormal range; the 8x cancels via exp-scale (CINV/8) on the
    score side and via rep = 1/(8*den) on the PV side.

Per-sample PE work drops 82944 -> ~50200 col-cycles (scores 16384 + PV-DR
16384 + qc 8192 + vt' 8192 + den 1024): every matmul issues moving columns at
1/cycle regardless of dtype, so fp8 DoubleRow's 2x comes purely from halving
the pass count (contraction 256 rows/pass).

  - est = exp(s*c - 4.5) stored fp8e5 (e5m2: scores empirically reach 13.8
    sigma, far past fp8e4's e^11.7 dynamic range; e5m2 spans it easily and
    softmax normalization cancels most of its 2-mantissa-bit noise -- host
    emulation: 0.0078 rel err vs 0.0068 for bf16 est). The offset cancels
    exactly in softmax. PV runs fp8 DoubleRow (e4m3 vt x e5m2 est) over
    jm-pair passes.

Hardware scheduling lessons baked in (each measured on traces):
  - ACT and Sync have exec-queue depth 0 (strict head-of-line blocking):
    never give them an op that waits on another engine ahead of
    latency-critical work. DVE (depth 8) and PE absorb such inversions.
  - Each DMA transfer is serviced by ~one engine at ~21 B/ns and needs
    2KB-contiguous rows + full 128 partitions for full rate; parallelism
    = in-flight transfer count. x loads are 8x256KB transfers, gated by
    semaphores on GN progress so the wire stays exclusive to the next
    needed tensor.
  - PSUM pool rotations create hidden cross-engine serialization: the
    den-reduce psums get a dedicated pool (rsp) so qc(b+1) never chains
    behind recip(b); tmpp has 4 bufs so the GpSimd residual adds stay off
    the critical path into scores(b+1).
  - softmax denominator: DVE pairwise-add tree over the 8 fp8 exp tiles
    (bf16 out, 2x DVE rate), one 8.0-matrix bf16 matmul reduces partitions
    and replicates 8*den; rep = 1/(8 den) via reciprocal_approx_fast.
  - residual: tmp = PV_psum * rep on DVE, out = tmp + x on GpSimd (idle
    otherwise; DVE for the last sample to shorten the tail), DMA per half.
  - engine balance per sample: PE ~20.9us, DVE ~19.8us (GN stats/apply, den
    tree, recip, tmp), ACT ~17.4us (exp + qc/vt' psum->sbuf fp8 copies),
    GpSimd ~9us (residual adds + out DMA issue).
  - prologue: x[0] owns all three DMA rings before weights/x[1..3] queue up;
    GN(0) stats start per-tile as x[0] tiles land.
"""

import os
import sys

import numpy as np

sys.path.insert(0, "/opt/trn_rl_repo")

import ml_dtypes  # noqa: E402

import concourse.bass as bass  # noqa: E402
import concourse.tile as tile  # noqa: E402
from concourse import bacc, mybir  # noqa: E402

P = 128
B_FULL, C, H, W = 32, 512, 32, 32
HW = H * W            # 1024 tokens
N_CORES = 8
NB = B_FULL // N_CORES  # 4 samples per core
NT = C // P           # 4 channel tiles
NP = NT // 2          # 2 DoubleRow channel-tile pairs
NJ = HW // P          # 8 token tiles
NJP = NJ // 2         # 4 DoubleRow token-tile pairs
NGROUPS = 32
GS = C // NGROUPS     # 16 channels per group
EPS = 1e-6
CINV = float(C) ** -0.5
WSC = 8.0             # fused weights pre-scaled by 8 (fp8 subnormal guard)
EOFF = -4.5           # exp offset: est = exp(s*c - 4.5), cancels in softmax

f32 = mybir.dt.float32
bf16 = mybir.dt.bfloat16
f8 = mybir.dt.float8e4
f8e5 = mybir.dt.float8e5
ALU = mybir.AluOpType
ACT = mybir.ActivationFunctionType
DR = mybir.MatmulPerfMode.DoubleRow


def build_nc(zero_out_bias=True):
    """Build the single-core Bass graph (SPMD: same graph on all 8 cores).

    zero_out_bias: fused output bias bt' = bt + Wt^T bv is all-zero (true for
    this problem), so the residual add drops the bias column.
    """
    nc = bacc.Bacc("TRN2", target_bir_lowering=False, debug=False)

    x_d = nc.dram_tensor("x", [NB, C, HW], f32, kind="ExternalInput")
    wqc_d = nc.dram_tensor("wqc", [P, NT, C], f8, kind="ExternalInput")
    wvt_d = nc.dram_tensor("wvt", [P, NT, C], f8, kind="ExternalInput")
    # gn affine columns: [:, 0, :]=gamma, [:, 1, :]=beta
    gab_d = nc.dram_tensor("gn_ab", [P, 2, NT], f32, kind="ExternalInput")
    # block-diagonal group-average matrix: GG[k,p] = 1/16 iff k//16 == p//16
    gg_d = nc.dram_tensor("gg", [P, P], f32, kind="ExternalInput")
    if not zero_out_bias:
        btp_d = nc.dram_tensor("btp", [P, NT], f32, kind="ExternalInput")
    out_d = nc.dram_tensor("out", [NB, C, HW], f32, kind="ExternalOutput")

    with tile.TileContext(nc) as tc:
        with (
            tc.tile_pool(name="consts", bufs=1) as consts,
            tc.tile_pool(name="hp", bufs=2) as hp,
            tc.tile_pool(name="qcp", bufs=2) as qcp,
            tc.tile_pool(name="vtp", bufs=2) as vtp,
            tc.tile_pool(name="est", bufs=1) as estp,
            tc.tile_pool(name="sump", bufs=1) as sump,
            tc.tile_pool(name="repp", bufs=1) as repp,
            # tmpp=4: with only 2 bufs, tmp(cm3)'s alloc waits the GpSimd
            # residual-add of cm1, putting the slow GpSimd chain on the
            # critical path into scores(b+1) via the psb rotation (measured
            # ~5us/iter stall).
            tc.tile_pool(name="tmpp", bufs=4) as tmpp,
            tc.tile_pool(name="outp", bufs=4) as outp,
            tc.tile_pool(name="small", bufs=2) as small,
            # psb 2x[P,1024] (4) + psq 2x[P,512] (2) + rsp 2x[P,512] (2)
            # = all 8 banks. The den-reduce psums get their OWN pool (rsp):
            # inside the psq rotation, qc(b+1)'s first psum alloc waits on
            # recip(b), serializing the whole next-sample projection phase
            # behind the exp->gpsimd-tree->den-red->recip latency chain
            # (~10us/iter, measured).
            tc.tile_pool(name="psb", bufs=2, space="PSUM") as psb,
            tc.tile_pool(name="psq", bufs=2, space="PSUM") as psq,
            tc.tile_pool(name="rsp", bufs=2, space="PSUM") as rsp,
        ):
            x_sb = consts.tile([P, NB, NT, HW], f32, tag="x")
            wqc_sb = consts.tile([P, NT, C], f8, tag="wqc")
            wvt_sb = consts.tile([P, NT, C], f8, tag="wvt")
            gab_sb = consts.tile([P, 2, NT], f32, tag="gab")
            gg_sb = consts.tile([P, P], f32, tag="gg")
            if not zero_out_bias:
                btp_sb = consts.tile([P, NT], f32, tag="btp")

            # gg+gab first on gpsimd (tiny, needed by affine(0)); then x[0]
            # halves (2KB contiguous per partition row) own ALL THREE rings.
            # DMA engines pull from all queued transfers CONCURRENTLY (not
            # ring-order), so x[1..3] loads are semaphore-GATED behind GN
            # progress below -- otherwise they steal ~2/3 of the wire and
            # x[0] (which gates the whole GN(0)->qc(0) prologue chain) lands
            # ~7us late (measured).
            # Each DMA transfer is serviced by ~one engine at ~21 B/ns, so
            # wire parallelism = in-flight transfer count. Transfers must
            # keep 2KB-contiguous rows and FULL 128 partitions (64-partition
            # transfers run at half rate -- measured), so a 2MB sample maxes
            # out at 8 transfers = ~12us. x[1..3] are semaphore-gated behind
            # GN progress so they don't steal the wire from x[0].
            nc.gpsimd.dma_start(out=gg_sb[:, :], in_=gg_d[:, :])
            nc.gpsimd.dma_start(out=gab_sb[:, :, :], in_=gab_d[:, :, :])
            engs = (nc.sync, nc.scalar, nc.gpsimd)
            qi = 0
            for t in range(NT):
                for h0 in (0, 512):
                    engs[qi % 3].dma_start(
                        out=x_sb[:, 0, t, h0:h0 + 512],
                        in_=x_d[0, t * P:(t + 1) * P, h0:h0 + 512])
                    qi += 1
            # weights in 2 transfers each, right behind x[0]
            for t2 in (0, 2):
                nc.sync.dma_start(out=wqc_sb[:, t2:t2 + 2, :],
                                  in_=wqc_d[:, t2:t2 + 2, :])
                nc.scalar.dma_start(out=wvt_sb[:, t2:t2 + 2, :],
                                    in_=wvt_d[:, t2:t2 + 2, :])
            if not zero_out_bias:
                nc.gpsimd.dma_start(out=btp_sb[:, :], in_=btp_d[:, :])

            def load_x(b, eng_list, after):
                """Issue sample b's x as 8 half-tile transfers (full rate)
                round-robined over eng_list, gated behind `after`."""
                qi = 0
                for t in range(NT):
                    for h0 in (0, 512):
                        eng = eng_list[qi % len(eng_list)]
                        qi += 1
                        dd = eng.dma_start(
                            out=x_sb[:, b, t, h0:h0 + 512],
                            in_=x_d[b, t * P:(t + 1) * P, h0:h0 + 512])
                        if after is not None:
                            tile.add_dep_helper(dd.ins, after.ins, sync=True,
                                                reason="x load gated on GN progress")

            # 8.0-matrix: reduces den partials across partitions AND folds the
            # 8x fused-weight scale (rep = 1/(8*den))
            ones_sb = consts.tile([P, P], bf16, tag="ones")
            nc.vector.memset(ones_sb[:, :], WSC)
            magic_sb = consts.tile([P, NT], mybir.dt.int32, tag="magic")
            nc.vector.memset(magic_sb[:, :], 0x5F3759DF)
            # per-partition exp-offset column (bias operand for the Exp calls)
            eoff_sb = consts.tile([P, 1], f32, tag="eoff")
            nc.vector.memset(eoff_sb[:, :], EOFF)
            # dummy Exp: pulls the ACT Exp-table load into the idle prologue
            expwarm = small.tile([P, 1], f32, tag="expwarm")
            nc.scalar.activation(out=expwarm[:, :], in_=ones_sb[:, 0:1],
                                 func=ACT.Exp, scale=CINV / WSC,
                                 bias=eoff_sb[:, 0:1])

            a_all = consts.tile([P, NB, NT], f32, tag="a_all")
            b_all = consts.tile([P, NB, NT], f32, tag="b_all")

            # PE warm-up on gg (earliest-arriving tensor): releases the HAM
            # clock-gate before the real stream.
            warm_ps = psq.tile([P, 512], f32, tag="qkv")
            for w in range(5):
                nc.tensor.matmul(
                    warm_ps[:, 0:128], gg_sb[:, 0:128], gg_sb[:, 0:128],
                    start=(w == 0), stop=(w == 4),
                )

            def gn_stats(b, after=None, mv=None, ts=0, te=NT):
                """bn stats -> per-channel (mean, Ex2) in mv[:, ts:te, :].
                GN groups (16ch) never span 128-channel tiles, so the whole
                GN chain is per-tile independent -- the prologue exploits
                this with 2-tile batches that chase the x DMAs."""
                if mv is None:
                    mv = small.tile([P, NT, 2], f32, tag="mv")
                nsub = 2
                step = HW // nsub
                first = None
                for t in range(ts, te):
                    st6 = small.tile([P, nsub, 6], f32, tag="st6")
                    for q in range(nsub):
                        iq = nc.vector.bn_stats(
                            out=st6[:, q, :],
                            in_=x_sb[:, b, t, q * step:(q + 1) * step])
                        if first is None:
                            first = iq
                        if after is not None:
                            tile.add_dep_helper(iq.ins, after.ins, sync=False,
                                                reason="gn stats after prev apply")
                    nc.vector.bn_aggr(out=mv[:, t, :], in_=st6[:, :, :])
                msq = small.tile([P, NT], f32, tag="msq")
                nc.vector.tensor_mul(msq[:, ts:te], mv[:, ts:te, 0],
                                     mv[:, ts:te, 0])
                nc.vector.tensor_add(mv[:, ts:te, 1], mv[:, ts:te, 1],
                                     msq[:, ts:te])
                return mv, first

            def gn_affine(b, mv, ts=0, te=NT):
                """fused group-avg+broadcast matmul, then per-channel A/B
                for tiles [ts, te)."""
                nw = (te - ts) * 2
                bc_ps = psq.tile([P, 512], f32, tag="qkv")
                nc.tensor.matmul(bc_ps[:, :nw], gg_sb[:, :], mv[:, ts:te, :],
                                 start=True, stop=True)
                bc = small.tile([P, NT, 2], f32, tag="bcs")
                nc.vector.tensor_copy(bc[:, ts:te, :], bc_ps[:, 0:nw])
                vb = small.tile([P, NT], f32, tag="vb")
                nc.vector.tensor_mul(vb[:, ts:te], bc[:, ts:te, 0], bc[:, ts:te, 0])
                nc.vector.tensor_sub(vb[:, ts:te], bc[:, ts:te, 1], vb[:, ts:te])
                nc.vector.tensor_scalar_add(vb[:, ts:te], vb[:, ts:te], EPS)
                # rstd = rsqrt(var+eps): fast-inverse-sqrt + 1 Newton step
                # (all-DVE: keeps Sqrt off ACT so it never evicts Exp)
                ii = small.tile([P, NT], mybir.dt.int32, tag="ii")
                nc.vector.tensor_scalar(
                    out=ii[:, ts:te], in0=vb.bitcast(mybir.dt.int32)[:, ts:te],
                    scalar1=1, scalar2=None, op0=ALU.arith_shift_right)
                nc.vector.tensor_tensor(ii[:, ts:te], magic_sb[:, ts:te],
                                        ii[:, ts:te], op=ALU.subtract)
                y0 = ii.bitcast(f32)
                yt = small.tile([P, NT], f32, tag="yt")
                y1 = small.tile([P, NT], f32, tag="y1")
                nc.vector.tensor_mul(yt[:, ts:te], vb[:, ts:te], y0[:, ts:te])
                nc.vector.tensor_mul(yt[:, ts:te], yt[:, ts:te], y0[:, ts:te])
                nc.vector.tensor_scalar(out=yt[:, ts:te], in0=yt[:, ts:te],
                                        scalar1=-0.5, scalar2=1.5,
                                        op0=ALU.mult, op1=ALU.add)
                nc.vector.tensor_mul(y1[:, ts:te], y0[:, ts:te], yt[:, ts:te])
                tmp = small.tile([P, NT], f32, tag="tmpab")
                nc.vector.tensor_mul(a_all[:, b, ts:te], y1[:, ts:te],
                                     gab_sb[:, 0, ts:te])
                nc.vector.tensor_mul(tmp[:, ts:te], bc[:, ts:te, 0],
                                     a_all[:, b, ts:te])
                nc.vector.tensor_sub(b_all[:, b, ts:te], gab_sb[:, 1, ts:te],
                                     tmp[:, ts:te])

            def apply_h(b, after=None, h=None, ts=0, te=NT, act01=False):
                """h = x*A + B (fp8). act01: tiles 0-1 on ACT (Identity with
                per-partition scale+bias) -- used for the IN-LOOP applies,
                where ACT is idle between the vt copies and the next exp
                batch while the DVE tail (which gates qc(b+2) via apply-t1,
                ~5us/iter measured) lags. Prologue applies stay all-DVE:
                there ACT's depth-0 queue would head-block exp(0)."""
                if h is None:
                    h = hp.tile([P, NT, HW], f8, tag="h")
                last = None
                for t in range(ts, te):
                    if act01 and t < 2:
                        li = nc.scalar.activation(
                            out=h[:, t, :], in_=x_sb[:, b, t, :],
                            func=ACT.Identity,
                            scale=a_all[:, b, t:t + 1],
                            bias=b_all[:, b, t:t + 1],
                        )
                    else:
                        li = nc.vector.tensor_scalar(
                            out=h[:, t, :], in0=x_sb[:, b, t, :],
                            scalar1=a_all[:, b, t:t + 1],
                            scalar2=b_all[:, b, t:t + 1],
                            op0=ALU.mult, op1=ALU.add,
                        )
                        last = li
                    if after is not None:
                        tile.add_dep_helper(li.ins, after.ins, sync=False,
                                            reason="apply after prev tmp")
                return h, last

            def gn_full_batched(b, after=None):
                """prologue GN in two 2-tile batches so apply(t0,t1) lands
                as soon as those x tiles arrive (qc's first DR pass only
                needs h tiles 0-1)."""
                mv = small.tile([P, NT, 2], f32, tag="mv")
                h = hp.tile([P, NT, HW], f8, tag="h")
                mv, first = gn_stats(b, after=after, mv=mv, ts=0, te=2)
                gn_affine(b, mv, 0, 2)
                _, last = apply_h(b, h=h, ts=0, te=2)
                gn_stats(b, mv=mv, ts=2, te=NT)
                gn_affine(b, mv, 2, NT)
                _, last = apply_h(b, h=h, ts=2, te=NT)
                return h, last, first

            def qc_proj(b, h_sb):
                """qc = A^T h (fp8 [d, hw], 8x-scaled); psum->sbuf on ACT."""
                qc_sb = qcp.tile([P, NT, HW], f8, tag="qc")
                for dm in range(NT):
                    ps0 = psq.tile([P, 512], f32, tag="qkv")
                    ps1 = psq.tile([P, 512], f32, tag="qkv")
                    for kp in range(NP):
                        lhsT = wqc_sb[:, 2 * kp:2 * kp + 2, dm * P:(dm + 1) * P]
                        st, sp = (kp == 0), (kp == NP - 1)
                        nc.tensor.matmul(ps0[:, :], lhsT,
                                         h_sb[:, 2 * kp:2 * kp + 2, 0:512],
                                         start=st, stop=sp, perf_mode=DR)
                        nc.tensor.matmul(ps1[:, :], lhsT,
                                         h_sb[:, 2 * kp:2 * kp + 2, 512:1024],
                                         start=st, stop=sp, perf_mode=DR)
                    nc.scalar.copy(qc_sb[:, dm, 0:512], ps0[:, :])
                    nc.scalar.copy(qc_sb[:, dm, 512:1024], ps1[:, :])
                return qc_sb

            def vt_proj(b, h_sb, split_copies=False):
                """vt' = (Wv Wt)^T h, transposed [hw, d] fp8 (8x-scaled);
                psum->sbuf on ACT (alternating with DVE for the deferred
                vt'(0), whose copies would otherwise stack up behind qc(1)'s
                in iteration 0's ACT queue)."""
                vt_sb = vtp.tile([P, NJ, C], f8, tag="vt")
                for jm in range(NJ):
                    ps = psq.tile([P, 512], f32, tag="qkv")
                    for kp in range(NP):
                        nc.tensor.matmul(
                            ps[:, :],
                            h_sb[:, 2 * kp:2 * kp + 2, jm * P:(jm + 1) * P],
                            wvt_sb[:, 2 * kp:2 * kp + 2, :],
                            start=(kp == 0), stop=(kp == NP - 1), perf_mode=DR,
                        )
                    if split_copies and jm % 2 == 0:
                        nc.vector.tensor_copy(vt_sb[:, jm, :], ps[:, :])
                    else:
                        nc.scalar.copy(vt_sb[:, jm, :], ps[:, :])
                return vt_sb

            # ---------------- prologue ----------------
            # vt'(0) is NOT issued here: it would delay scores(0) by 3.4us of
            # PE time; it slots into iteration 0 between qc(1) and vt'(1),
            # where its ACT copies run after exp(0) drains.
            # x[1] loads fire once GN(0) is underway (x[0] fully landed soon
            # after); x[2]/x[3] once GN(1) starts: keeps the wire exclusive
            # to whatever the pipeline needs next. (GN is NOT tile-batched:
            # the affine chain is small-op-overhead-dominated, and doubling
            # it costs more than the earlier apply saves -- measured.)
            mv0, stats0_first = gn_stats(0)
            load_x(1, (nc.sync, nc.scalar), stats0_first)
            gn_affine(0, mv0)
            h0_sb, last_apply = apply_h(0)

            qc_next = qc_proj(0, h0_sb)

            mv1, stats1_first = gn_stats(1, after=last_apply)
            load_x(2, (nc.gpsimd, nc.sync), stats1_first)
            load_x(3, (nc.sync, nc.gpsimd), stats1_first)
            gn_affine(1, mv1)
            h_next, last_apply = apply_h(1)

            vt_next = None
            h_cur = h0_sb
            last_tmp = None

            # ---------------- main loop ----------------
            mv_next = None
            for b in range(NB):
                qc_sb = qc_next
                vt_sb = vt_next

                # iter>=1: GN stats for b+2 at the DVE-queue HEAD -- its x
                # landed long ago, it has no cross-engine deps, and it fills
                # the DVE idle while exp(b) produces the den-tree inputs.
                if b >= 1 and b + 2 < NB:
                    mv_next, _ = gn_stats(b + 2, after=last_apply)

                # ---- scores transposed (ST[j,i] = sum_d h[d,j] qc[d,i])
                #      + exp (ACT, fp8 out with -2 offset) ----
                est_sb = estp.tile([P, NJ, HW], f8e5, tag="est")
                for jm in range(NJ):
                    st_ps = psb.tile([P, HW], f32, tag="big")
                    for kp in range(NP):
                        lhsT = h_cur[:, 2 * kp:2 * kp + 2, jm * P:(jm + 1) * P]
                        st, sp = (kp == 0), (kp == NP - 1)
                        nc.tensor.matmul(st_ps[:, 0:512], lhsT,
                                         qc_sb[:, 2 * kp:2 * kp + 2, 0:512],
                                         start=st, stop=sp, perf_mode=DR)
                        nc.tensor.matmul(st_ps[:, 512:1024], lhsT,
                                         qc_sb[:, 2 * kp:2 * kp + 2, 512:1024],
                                         start=st, stop=sp, perf_mode=DR)
                    nc.scalar.activation(
                        out=est_sb[:, jm, :], in_=st_ps[:, :], func=ACT.Exp,
                        scale=CINV / WSC, bias=eoff_sb[:, 0:1],
                    )

                # ---- softmax denominator tree (bf16 out): level-1 split
                # 2 adds DVE / 2 adds GpSimd (idle early-iteration) ----
                s0 = sump.tile([P, HW], bf16, tag="s0")
                s1 = sump.tile([P, HW], bf16, tag="s1")
                s2 = sump.tile([P, HW], bf16, tag="s2")
                s3 = sump.tile([P, HW], bf16, tag="s3")
                nc.vector.tensor_add(s0[:, :], est_sb[:, 0, :], est_sb[:, 1, :])
                nc.gpsimd.tensor_add(s1[:, :], est_sb[:, 2, :], est_sb[:, 3, :])
                nc.vector.tensor_add(s2[:, :], est_sb[:, 4, :], est_sb[:, 5, :])
                nc.gpsimd.tensor_add(s3[:, :], est_sb[:, 6, :], est_sb[:, 7, :])
                nc.vector.tensor_add(s0[:, :], s0[:, :], s1[:, :])
                nc.vector.tensor_add(s2[:, :], s2[:, :], s3[:, :])
                nc.vector.tensor_add(s0[:, :], s0[:, :], s2[:, :])

                # ---- qc, vt' for sample b+1 fill TensorE while ACT exps ----
                if b + 1 < NB:
                    qc_next = qc_proj(b + 1, h_next)
                if b == 0:
                    vt_sb = vt_proj(0, h0_sb)
                if b + 1 < NB:
                    vt_next = vt_proj(b + 1, h_next)

                # ---- PV fp8 DoubleRow over jm pairs: psum IS the final
                #      pre-residual activation (out-proj fused into vt').
                #      The den partition-reduce slots in AFTER PV cm0: rep is
                #      first needed by tmp(cm0), so this buys the DVE den
                #      tree ~2.3us of extra slack before the PE blocks on it.
                last_tmp = None
                rep = repp.tile([P, HW], f32, tag="rep")
                last_sample = b == NB - 1

                def den_recip():
                    rs0 = rsp.tile([P, 512], f32, tag="rs")
                    rs1 = rsp.tile([P, 512], f32, tag="rs")
                    nc.tensor.matmul(rs0[:, :], ones_sb[:, :], s0[:, 0:512],
                                     start=True, stop=True)
                    nc.tensor.matmul(rs1[:, :], ones_sb[:, :],
                                     s0[:, 512:1024], start=True, stop=True)
                    nc.vector.reciprocal_approx_fast(out=rep[:, 0:512],
                                                     in_=rs0[:, :])
                    nc.vector.reciprocal_approx_fast(out=rep[:, 512:1024],
                                                     in_=rs1[:, :])

                for cm in range(NT):
                    o_ps = psb.tile([P, HW], f32, tag="big")
                    for t in range(NJP):
                        lhsT = vt_sb[:, 2 * t:2 * t + 2, cm * P:(cm + 1) * P]
                        st, sp = (t == 0), (t == NJP - 1)
                        nc.tensor.matmul(o_ps[:, 0:512], lhsT,
                                         est_sb[:, 2 * t:2 * t + 2, 0:512],
                                         start=st, stop=sp, perf_mode=DR)
                        nc.tensor.matmul(o_ps[:, 512:1024], lhsT,
                                         est_sb[:, 2 * t:2 * t + 2, 512:1024],
                                         start=st, stop=sp, perf_mode=DR)
                    if cm == 0:
                        # den partition-reduce after PV cm0 (see note above)
                        den_recip()
                    # tmp = psum * rep on DVE; residual add on GpSimd
                    # (DVE for the last sample: shorter tail); DMA issues on
                    # the idle Sync engine (~600ns engine time each)
                    tmv = tmpp.tile([P, HW], f32, tag="tmp")
                    last_tmp = nc.vector.tensor_mul(tmv[:, :], o_ps[:, :],
                                                    rep[:, :])
                    out_t = outp.tile([P, HW], f32, tag="out")
                    for hh in (0, 512):
                        if zero_out_bias:
                            eng = nc.vector if last_sample else nc.gpsimd
                            eng.tensor_add(out_t[:, hh:hh + 512],
                                           tmv[:, hh:hh + 512],
                                           x_sb[:, b, cm, hh:hh + 512])
                        else:
                            nc.vector.scalar_tensor_tensor(
                                out=out_t[:, hh:hh + 512],
                                in0=tmv[:, hh:hh + 512],
                                scalar=btp_sb[:, cm:cm + 1],
                                in1=x_sb[:, b, cm, hh:hh + 512],
                                op0=ALU.add, op1=ALU.add,
                            )
                        deng = nc.scalar if (last_sample and hh) else nc.sync
                        deng.dma_start(
                            out=out_d[b, cm * P:(cm + 1) * P, hh:hh + 512],
                            in_=out_t[:, hh:hh + 512])

                # tail: stats(2) on iter-0 (x[2] lands mid-iter-0), then
                # affine(b+2) + apply(b+2); the bc matmul runs after PV(b).
                if b + 2 < NB:
                    h_cur = h_next
                    if b == 0:
                        mv_next, _ = gn_stats(b + 2, after=last_apply)
                    gn_affine(b + 2, mv_next)
                    h_next, last_apply = apply_h(b + 2, after=last_tmp,
                                                 act01=True)
                else:
                    h_cur = h_next

    nc.compile()
    return nc


def prep_inputs(inputs):
    """Host-side prep: per-core in_maps with fused, pre-laid-out weights."""
    e4 = ml_dtypes.float8_e4m3
    x = np.ascontiguousarray(np.asarray(inputs["x"], dtype=np.float32)).reshape(
        B_FULL, C, HW
    )

    Wq = np.asarray(inputs["Wq"], dtype=np.float64)
    Wk = np.asarray(inputs["Wk"], dtype=np.float64)
    Wv = np.asarray(inputs["Wv"], dtype=np.float64)
    Wt = np.asarray(inputs["Wt"], dtype=np.float64)
    A = (WSC * (Wq @ Wk.T)).astype(np.float32)   # scores = h^T A h / 8
    Bm = (WSC * (Wv @ Wt)).astype(np.float32)    # out = (B^T h)^T P / 8

    def wprep(w):
        # [C, C] -> [P, NT, C]  (lhsT slices w[kc*128+p, d])
        return np.ascontiguousarray(
            np.asarray(w, dtype=np.float32).reshape(NT, P, C).transpose(1, 0, 2)
        ).astype(e4)

    def cols(v):
        # [C] -> [P, NT]
        return np.ascontiguousarray(
            np.asarray(v, dtype=np.float32).reshape(NT, P).T
        )

    gab = np.stack([cols(inputs["gn_scale"]), cols(inputs["gn_bias"])], axis=1)
    gg = np.zeros((P, P), np.float32)
    for p in range(P):
        gg[p, (p // GS) * GS:(p // GS + 1) * GS] = 1.0 / GS

    btp = (np.asarray(inputs["bt"], dtype=np.float64)
           + Wt.T @ np.asarray(inputs["bv"], dtype=np.float64)).astype(np.float32)

    shared = {
        "wqc": wprep(A), "wvt": wprep(Bm),
        "gn_ab": np.ascontiguousarray(gab), "gg": gg,
    }
    if np.any(btp != 0):
        shared["btp"] = cols(btp)
    in_maps = []
    for c_id in range(N_CORES):
        m = dict(shared)
        m["x"] = np.ascontiguousarray(x[c_id * NB:(c_id + 1) * NB])
        in_maps.append(m)
    return in_maps


_NC_CACHE = {}


def get_nc(zero_out_bias=True):
    key = (zero_out_bias,)
    if key not in _NC_CACHE:
        _NC_CACHE[key] = build_nc(zero_out_bias=zero_out_bias)
    return _NC_CACHE[key]


def _kernel_numpy(inputs):
    """Pure-numpy fallback, only for nonzero bq/bk (never hit by this
    problem's inputs -- setup_inputs() uses zero biases)."""
    x = np.asarray(inputs["x"], dtype=np.float64)
    B, C_, H_, W_ = x.shape
    g = x.reshape(B, NGROUPS, C_ // NGROUPS, H_, W_)
    mean = g.mean(axis=(2, 3, 4), keepdims=True)
    var = g.var(axis=(2, 3, 4), keepdims=True)
    hn = ((g - mean) / np.sqrt(var + EPS)).reshape(B, C_, H_, W_)
    hn = hn * np.asarray(inputs["gn_scale"], np.float64)[None, :, None, None] \
        + np.asarray(inputs["gn_bias"], np.float64)[None, :, None, None]

    def nin(h, Wm, bm):
        return np.einsum("bchw,cd->bdhw", h, np.asarray(Wm, np.float64)) \
            + np.asarray(bm, np.float64)[None, :, None, None]

    q = nin(hn, inputs["Wq"], inputs["bq"])
    k = nin(hn, inputs["Wk"], inputs["bk"])
    v = nin(hn, inputs["Wv"], inputs["bv"])
    w = np.einsum("bchw,bcij->bhwij", q, k) * (C_ ** -0.5)
    w = w.reshape(B, H_, W_, H_ * W_)
    w = np.exp(w - w.max(axis=-1, keepdims=True))
    w = (w / w.sum(axis=-1, keepdims=True)).reshape(B, H_, W_, H_, W_)
    hh = np.einsum("bhwij,bcij->bchw", w, v)
    hh = nin(hh, inputs["Wt"], inputs["bt"])
    return (hh + x).astype(np.float32)


def run(inputs, trace=False):
    from concourse.bass_utils import run_bass_kernel_spmd

    if not (np.all(np.asarray(inputs["bq"]) == 0)
            and np.all(np.asarray(inputs["bk"]) == 0)):
        return _kernel_numpy(inputs), None

    in_maps = prep_inputs(inputs)
    nc = get_nc(zero_out_bias="btp" not in in_maps[0])
    res = run_bass_kernel_spmd(
        nc, in_maps, core_ids=list(range(N_CORES)), trace=trace
    )
    out = np.concatenate([np.asarray(r["out"]) for r in res.results], axis=0)
    return out.reshape(B_FULL, C, H, W), res


def kernel(**inputs):
    out, _ = run(inputs, trace=False)
    return out
